# revision 33
# baseline (speedup 1.0000x reference)
"""FRFN forward kernel for 8 Trainium2 NeuronCores.

Sharding: pure data parallel over batch B=64 -> 8 batches per core.
The TVConv generated weight (batch-independent) is recomputed on every
core.

Per-core pipeline (channel dims padded so x1/x2 halves align at 768):
  weightgen: 3x (3x3 conv + LayerNorm(CHW) + relu) on 4x14x14 posi map
             then final conv -> wgt[ct] = (128, 9, 196) bf16 per tile
  proj_in  : h = W_in @ x        PE bf16 -> padded (128, 8, 16, 16)
  tvconv   : prod_k = wgt_k * h_win_k   DVE (6 taps) + Pool (3 taps)
             partial pre-adds on DVE, remaining streams summed on the
             PE as identity-matmul PSUM accumulation
  gate     : x1 tiles: gelu straight from PSUM (ACT)
             x2 tiles: gated = ga * psum  (DVE, mixed dtype)
  proj_out : W_out @ gated               PE bf16
"""

import numpy as np
import ml_dtypes
from contextlib import ExitStack

import concourse.bacc as bacc
import concourse.bass as bass
import concourse.mybir as mybir
import concourse.tile as tile
from concourse.bass_utils import run_bass_kernel_spmd

F32 = mybir.dt.float32
BF16 = mybir.dt.bfloat16
AF = mybir.ActivationFunctionType
OP = mybir.AluOpType

NCORES = 8
B = 64
BPC = B // NCORES          # 8 batches per core
DIM = 256
HID = 680
CH = 2 * HID               # 1360
HIDP = 768                 # padded x1/x2 half (6 * 128)
CHP = 2 * HIDP             # 1536
NCT = CHP // 128           # 12 channel tiles
NGT = HIDP // 128          # 6 gate tiles
HP = 14
NIJ = HP * HP              # 196
PH = 16                    # padded spatial side
INTER = 64
NKPL = 9                   # 3x3 taps
KT_ROWS = [128, 128, 128, 128, 64]   # 576 contraction rows
NCHUNK = 4                 # 392-col psum chunks (2 batches x 196)
NB2 = 2 * NIJ              # 392
EPS = 1e-5
NLN = float(INTER * NIJ)

# channel-tile visit order: gate pairs adjacent (x1 then its x2)
CT_ORDER = [0, 6, 1, 7, 2, 8, 3, 9, 4, 10, 5, 11]
# taps computed on the Pool engine (slow, so they stream last)
POOL_TAPS = (6, 7, 8)
# pre-add pairs on DVE; streams ordered by readiness, the last one (tap 8,
# Pool's final product) is closed at the start of the NEXT stage
PRE_A3x = [(0, 1), (2, 3), (4, 5)]
STREAMS3 = [6, 7, 0, 2, 4, 8]
PRE_A4x = [(0, 1), (2, 3), (4, 5), (6, 7)]
STREAMS4 = [0, 2, 4, 6, 8]

_CACHE = {}


def _build_nc():
    nc = bacc.Bacc("TRN2", target_bir_lowering=False)

    xT = nc.dram_tensor("xT", [128, 2 * BPC * NIJ], BF16,
                        kind="ExternalInput")
    winT = nc.dram_tensor("winT", [128, 2 * CHP], BF16,
                          kind="ExternalInput")
    # first conv folded host-side: posi_map is all-ones, so conv0 reduces to
    # (sum_cin w0)[tap, c] @ border_mask[tap, ij]
    w0S = nc.dram_tensor("w0S", [NKPL, INTER], BF16, kind="ExternalInput")
    bmask = nc.dram_tensor("bmask", [NKPL, NIJ], BF16, kind="ExternalInput")
    w1T = nc.dram_tensor("w1T", [INTER, NKPL, INTER], BF16,
                         kind="ExternalInput")
    w2T = nc.dram_tensor("w2T", [INTER, NKPL, INTER], BF16,
                         kind="ExternalInput")
    wfT = nc.dram_tensor("wfT", [128, NCT * 5 * NKPL * 128], BF16,
                         kind="ExternalInput")
    woutT = nc.dram_tensor("woutT", [128, NGT * DIM], BF16,
                           kind="ExternalInput")
    identD = nc.dram_tensor("identD", [128, 128], BF16, kind="ExternalInput")
    out_f = nc.dram_tensor("out_f", [DIM, BPC * NIJ], F32,
                           kind="ExternalOutput")

    with tile.TileContext(nc) as tc, ExitStack() as ctx:
        persist = ctx.enter_context(tc.tile_pool(name="persist", bufs=1))
        work = ctx.enter_context(tc.tile_pool(name="work", bufs=2))
        wgtpool = ctx.enter_context(tc.tile_pool(name="wgtpool", bufs=2))
        prodpool = ctx.enter_context(tc.tile_pool(name="prodpool", bufs=1))
        gapool = ctx.enter_context(tc.tile_pool(name="gapool", bufs=2))
        wfpool = ctx.enter_context(tc.tile_pool(name="wfpool", bufs=2))
        # 4 rolling single-bank psum units + one 4-bank accumulator
        ps_u = ctx.enter_context(
            tc.tile_pool(name="ps_u", bufs=4, space="PSUM"))
        ps_pst = ctx.enter_context(
            tc.tile_pool(name="ps_pst", bufs=1, space="PSUM"))

        # ---------------- persistent SBUF tensors ----------------
        h_sb = [persist.tile([128, BPC, PH, PH], BF16, name="t", tag=f"h{i}")
                for i in range(NCT)]
        gated = [persist.tile([128, BPC * NIJ], BF16, name="t", tag=f"gd{i}")
                 for i in range(NGT)]
        wout_sb = persist.tile([128, NGT, DIM], BF16, name="t", tag="wo")
        x_sb = persist.tile([128, 2, BPC * NIJ], BF16, name="t", tag="x")
        win_sb = persist.tile([128, 2, CHP], BF16, name="t", tag="wi")

        w0_sb = persist.tile([NKPL, INTER], BF16, name="t", tag="w0")
        bm_sb = persist.tile([NKPL, NIJ], BF16, name="t", tag="bm")
        w1_sb = persist.tile([INTER, NKPL, INTER], BF16, name="t", tag="w1")
        w2_sb = persist.tile([INTER, NKPL, INTER], BF16, name="t", tag="w2")
        pad1 = persist.tile([INTER, PH, PH], BF16, name="t", tag="pad1")
        pad2 = persist.tile([INTER, PH, PH], BF16, name="t", tag="pad2")
        pad3 = persist.tile([INTER, PH, PH], BF16, name="t", tag="pad3")
        p3 = [persist.tile([128, NIJ], BF16, name="t", tag=f"p3_{k}")
              for k in range(5)]
        ones_b = persist.tile([INTER, INTER], F32, name="t", tag="ones_b")
        ident = persist.tile([128, 128], BF16, name="t", tag="ident")
        eps_v = persist.tile([INTER, 1], F32, name="t", tag="eps")

        # ---------------- input DMAs + memsets ----------------
        # tiny LN-chain inputs first (unblock the serial head chain), then
        # ident, x/win; wf streams next (emitted below); wout last (tail)
        nc.sync.dma_start(w0_sb[:], w0S[:])
        nc.sync.dma_start(bm_sb[:], bmask[:])
        nc.sync.dma_start(w1_sb[:], w1T[:])
        nc.sync.dma_start(w2_sb[:], w2T[:])
        nc.sync.dma_start(ident[:], identD[:])
        nc.sync.dma_start(x_sb[:].rearrange("p a b -> p (a b)"), xT[:])
        nc.sync.dma_start(win_sb[:].rearrange("p a b -> p (a b)"), winT[:])
        nc.sync.dma_start(wout_sb[:].rearrange("p a b -> p (a b)"), woutT[:])

        # pre-warm the sqrt ACT table (LN chain); the gelu table is warmed
        # by a dummy activation right after the chain so the 1.3us table
        # load lands off the critical path
        warm = persist.tile([1, 1], F32, name="t", tag="warm")
        wsink = persist.tile([1, 1], F32, name="t", tag="wsink")
        nc.gpsimd.memset(warm[:], 1.0)
        nc.scalar.activation(wsink[:], warm[:], AF.Sqrt)

        nc.gpsimd.memset(ones_b[:], 1.0)
        nc.gpsimd.memset(eps_v[:], EPS)
        nc.gpsimd.memset(pad1[:], 0.0)
        nc.gpsimd.memset(pad2[:], 0.0)
        nc.gpsimd.memset(pad3[:], 0.0)
        nc.vector.memset(p3[4][64:128, :], 0.0)
        for i in CT_ORDER:
            # zero only the pad borders (proj_in drains fill the interior);
            # on DVE, idle during the head
            t = h_sb[i]
            nc.vector.memset(t[:, :, 0, :], 0.0)
            nc.vector.memset(t[:, :, 15, :], 0.0)
            nc.vector.memset(t[:, :, 1:15, 0], 0.0)
            nc.vector.memset(t[:, :, 1:15, 15], 0.0)

        # ------------- weight-gen small conv chain (fp32) -------------
        # LayerNorm([C,H,W]) with the reference's g=1, b=0 folded out.
        # Stats are reduced AND broadcast in one matmul with an all-ones
        # [64,64] stationary, so every partition holds [sum, sumsq] and the
        # normalize reads the conv result straight from PSUM.
        def layernorm_relu(ps_in, pad_tile):
            sq = work.tile([INTER, NIJ], F32, name="t", tag="ln_sq")
            stats = work.tile([INTER, 2], F32, name="t", tag="ln_st")
            nc.scalar.activation(sq[:], ps_in, AF.Square,
                                 accum_out=stats[:, 1:2])
            nc.vector.tensor_reduce(stats[:, 0:1], ps_in,
                                    mybir.AxisListType.X, OP.add)
            ps_bc = ps_u.tile([INTER, 2], F32, name="t", tag="u")
            nc.tensor.matmul(ps_bc[:], ones_b[:], stats[:],
                             start=True, stop=True)
            mr2 = work.tile([INTER, 2], F32, name="t", tag="ln_mr")
            nc.vector.tensor_scalar_mul(mr2[:], ps_bc[:], 1.0 / NLN)
            musq = work.tile([INTER, 1], F32, name="t", tag="ln_musq")
            nc.vector.tensor_mul(musq[:], mr2[:, 0:1], mr2[:, 0:1])
            var = work.tile([INTER, 1], F32, name="t", tag="ln_var")
            nc.vector.tensor_sub(var[:], mr2[:, 1:2], musq[:])
            std = work.tile([INTER, 1], F32, name="t", tag="ln_std")
            nc.scalar.activation(std[:], var[:], AF.Sqrt, bias=eps_v[:])
            rstd = work.tile([INTER, 1], F32, name="t", tag="ln_rstd")
            nc.vector.reciprocal(rstd[:], std[:])
            xn = work.tile([INTER, NIJ], F32, name="t", tag="ln_xn")
            nc.vector.tensor_scalar(xn[:], ps_in, mr2[:, 0:1], rstd[:],
                                    op0=OP.subtract, op1=OP.mult)
            dst = pad_tile[:, 1:15, 1:15]
            src = xn[:].rearrange("p (i j) -> p i j", i=HP, j=HP)
            nc.scalar.activation(dst, src, AF.Relu)

        def conv3x3(w_sb, pad_tile, ps_out):
            for kap in range(NKPL):
                di, dj = kap // 3, kap % 3
                nc.tensor.matmul(ps_out, w_sb[:, kap, :],
                                 pad_tile[:, di:di + HP, dj:dj + HP],
                                 start=(kap == 0), stop=(kap == NKPL - 1))

        # ---------------- per-stage emission helpers ----------------
        def emit_proj_in(ct):
            """proj_in for one channel tile -> h_sb[ct] (padded layout)."""
            for ch in range(NCHUNK):
                u = ps_u.tile([128, NB2], F32, name="t", tag="u")
                for kt in range(2):
                    nc.tensor.matmul(
                        u[:],
                        win_sb[:, kt, 128 * ct:128 * (ct + 1)],
                        x_sb[:, kt, NB2 * ch:NB2 * (ch + 1)],
                        start=(kt == 0), stop=(kt == 1))
                dst = h_sb[ct][:, 2 * ch:2 * ch + 2, 1:15, 1:15]
                src = u[:].rearrange("p (b i j) -> p b i j", b=2, i=HP, j=HP)
                nc.scalar.activation(dst, src, AF.Copy)

        def emit_wf_load(ct):
            t = wfpool.tile([128, 5, NKPL * 128], BF16, name="t", tag="wf")
            c0 = 5 * NKPL * 128 * ct
            nc.sync.dma_start(t[:].rearrange("p a b -> p (a b)"),
                              wfT[:, c0:c0 + 5 * NKPL * 128])
            return t

        def emit_conv_f(ct, wf_t):
            """final conv for one channel tile -> wgt (128, 9, 196) bf16.
            Two taps share one psum bank; drained in 2-tap batches."""
            wgt = wgtpool.tile([128, NKPL, NIJ], BF16, name="t", tag="wgt")
            for t0 in range(0, NKPL, 2):
                ntap = min(2, NKPL - t0)
                u = ps_u.tile([128, NB2], F32, name="t", tag="u")
                for sub in range(ntap):
                    kpl = t0 + sub
                    dst = u[:, NIJ * sub:NIJ * (sub + 1)]
                    for kt in range(5):
                        nc.tensor.matmul(
                            dst,
                            wf_t[:, kt, 128 * kpl:128 * (kpl + 1)],
                            p3[kt][:],
                            start=(kt == 0), stop=(kt == 4))
                nc.scalar.activation(
                    wgt[:, t0:t0 + ntap, :],
                    u[:, 0:NIJ * ntap].rearrange("p (t f) -> p t f", t=ntap),
                    AF.Copy)
            return wgt

        def emit_mult(ct, wgt, t, eng):
            di, dj = t // 3, t % 3
            wgb = (wgt[:, t, :].rearrange("p (i j) -> p i j", i=HP, j=HP)
                   .unsqueeze(1).broadcast_to((128, BPC, HP, HP)))
            hwin = h_sb[ct][:, :, di:di + HP, dj:dj + HP]
            prod = prodpool.tile([128, BPC * NIJ], BF16,
                                 name="t", tag=f"prod{t}")
            pr = prod[:].rearrange("p (b i j) -> p b i j",
                                   b=BPC, i=HP, j=HP)
            eng.tensor_mul(pr, hwin, wgb)
            return prod

        def emit_identity(pst, prod, start, stop):
            for ch in range(NCHUNK):
                nc.tensor.matmul(
                    pst[:, ch, 0:NB2], ident[:],
                    prod[:, NB2 * ch:NB2 * (ch + 1)],
                    start=start, stop=stop)

        def emit_gate(pend):
            """Chunked gate for a finished tile: frees pst incrementally."""
            pst_p, ct_p = pend
            if ct_p < NGT:
                # x1 tile: gelu straight from psum, per 392-col chunk
                ga = gapool.tile([128, BPC * NIJ], BF16, name="t",
                                 tag=f"ga{ct_p}")
                for c in range(NCHUNK):
                    nc.scalar.activation(
                        ga[:, NB2 * c:NB2 * (c + 1)],
                        pst_p[:, c, 0:NB2], AF.Gelu)
                ga_map[ct_p] = ga
            else:
                # x2 tile: gated = gelu(x1) * x2, in 784-col halves
                ga = ga_map[ct_p - NGT]
                for hh in range(2):
                    nc.vector.tensor_mul(
                        gated[ct_p - NGT][:]
                        .rearrange("p (c f) -> p c f", c=NCHUNK)
                        [:, 2 * hh:2 * hh + 2, :],
                        pst_p[:, 2 * hh:2 * hh + 2, 0:NB2],
                        ga[:].rearrange("p (c f) -> p c f", c=NCHUNK)
                        [:, 2 * hh:2 * hh + 2, :])

        # =================== program ===================
        # start streaming the first two tiles' conv-f weights right away
        wf0 = emit_wf_load(CT_ORDER[0])
        wf1 = emit_wf_load(CT_ORDER[1])

        ps0 = ps_u.tile([INTER, NIJ], F32, name="t", tag="u")
        # conv0 folded to a single matmul (posi_map == ones)
        nc.tensor.matmul(ps0[:], w0_sb[:], bm_sb[:], start=True, stop=True)
        layernorm_relu(ps0[:], pad1)

        # proj_in prologue interleaved with the LN chain
        emit_proj_in(CT_ORDER[0])

        ps1 = ps_u.tile([INTER, NIJ], F32, name="t", tag="u")
        conv3x3(w1_sb, pad1, ps1[:])
        layernorm_relu(ps1[:], pad2)

        emit_proj_in(CT_ORDER[1])

        ps2 = ps_u.tile([INTER, NIJ], F32, name="t", tag="u")
        conv3x3(w2_sb, pad2, ps2[:])
        layernorm_relu(ps2[:], pad3)

        # warm the gelu table now (off the critical path)
        nc.scalar.activation(wsink[:], warm[:], AF.Gelu)

        # im2col of pad3 for the final conv (576 contraction rows)
        qengs = [nc.sync, nc.scalar]
        for kt in range(5):
            nk = KT_ROWS[kt] // 64
            for sub in range(nk):
                kap = 2 * kt + sub
                di, dj = kap // 3, kap % 3
                srcw = pad3[:, di:di + HP, dj:dj + HP]
                dst = p3[kt][64 * sub:64 * (sub + 1), :]
                dst = dst.rearrange("p (i j) -> p i j", i=HP, j=HP)
                qengs[kap % 2].dma_start(dst, srcw)

        # conv-f runs one stage ahead of the tap loop so wgt is drained
        # before the stage's first DVE mult
        wgt_next = emit_conv_f(CT_ORDER[0], wf0)
        wf_next = wf1
        ga_map = {}
        pending_close = None   # (pst, last_prod, ct)
        for s, ct in enumerate(CT_ORDER):
            wgt = wgt_next

            # close the previous tile's identity group first thing (its last
            # product finished at the end of the previous stage)
            gate_pend = None
            if pending_close is not None:
                pst_p, last_p, ct_p = pending_close
                emit_identity(pst_p, last_p, start=False, stop=True)
                gate_pend = (pst_p, ct_p)
                pending_close = None

            if s + 1 < NCT:
                wgt_next = emit_conv_f(CT_ORDER[s + 1], wf_next)
            if s + 2 < NCT:
                wf_next = emit_wf_load(CT_ORDER[s + 2])
            if s + 3 < NCT:
                emit_proj_in(CT_ORDER[s + 3])

            # Pool taps first (slow), then DVE taps with the previous gate
            # interleaved so neither engine idles
            prods = {}
            for t in POOL_TAPS:
                prods[t] = emit_mult(ct, wgt, t, nc.gpsimd)
            for t in (0, 1, 2, 3):
                prods[t] = emit_mult(ct, wgt, t, nc.vector)
            if gate_pend is not None:
                emit_gate(gate_pend)
            for t in (4, 5):
                prods[t] = emit_mult(ct, wgt, t, nc.vector)

            preadds = PRE_A3x if s % 2 == 0 else PRE_A4x
            merged = set()
            for (a, b) in preadds:
                if (a, b) == (4, 5):
                    # one pre-add rides on Pool (scalar_tensor_tensor)
                    nc.gpsimd.scalar_tensor_tensor(
                        prods[a][:], prods[a][:], 1.0, prods[b][:],
                        op0=OP.mult, op1=OP.add)
                else:
                    nc.vector.tensor_add(prods[a][:], prods[a][:],
                                         prods[b][:])
                merged.add(b)
            streams = [prods[t] for t in STREAMS3 if t not in merged] \
                if s % 2 == 0 else [prods[t] for t in STREAMS4
                                    if t not in merged]

            pst = ps_pst.tile([128, NCHUNK, 512], F32, name="t", tag="pst")
            for si, prod in enumerate(streams[:-1]):
                emit_identity(pst, prod, start=(si == 0), stop=False)
            pending_close = (pst, streams[-1], ct)

        # epilogue: close and gate the last tile
        pst_p, last_p, ct_p = pending_close
        emit_identity(pst_p, last_p, start=False, stop=True)
        emit_gate((pst_p, ct_p))

        # ---------------- proj_out: W_out @ gated ----------------
        outpool = ctx.enter_context(tc.tile_pool(name="outpool", bufs=4))
        for m in range(2):
            for ch in range(NCHUNK):
                u = ps_u.tile([128, NB2], F32, name="t", tag="u")
                for kt in range(NGT):
                    nc.tensor.matmul(
                        u[:],
                        wout_sb[:, kt, 128 * m:128 * (m + 1)],
                        gated[kt][:, NB2 * ch:NB2 * (ch + 1)],
                        start=(kt == 0), stop=(kt == NGT - 1))
                ot = outpool.tile([128, NB2], F32, name="t", tag="ot")
                nc.scalar.activation(ot[:], u[:], AF.Copy)
                nc.sync.dma_start(
                    out_f[128 * m:128 * (m + 1), NB2 * ch:NB2 * (ch + 1)],
                    ot[:])

    nc.compile()
    return nc


def _pack_shared(inputs):
    """Pack the batch-independent tensors (host-side layout marshalling)."""
    W_in = np.asarray(inputs["W_in"], np.float32)
    W_out = np.asarray(inputs["W_out"], np.float32)
    w0 = np.asarray(inputs["w0"], np.float32)
    w1 = np.asarray(inputs["w1"], np.float32)
    w2 = np.asarray(inputs["w2"], np.float32)
    wf = np.asarray(inputs["wf"], np.float32)

    padc = np.arange(CH)
    padc = np.where(padc < HID, padc, padc + (HIDP - HID))

    winP = np.zeros((CHP, DIM), np.float32)
    winP[padc] = W_in
    winT = np.ascontiguousarray(
        winP.T.reshape(2, 128, CHP).transpose(1, 0, 2).reshape(128, 2 * CHP)
    ).astype(ml_dtypes.bfloat16)

    # conv0 on the all-ones posi_map folds to (sum_cin w0) x border-mask
    w0S = np.ascontiguousarray(
        w0.sum(axis=1).reshape(INTER, 9).T).astype(ml_dtypes.bfloat16)
    ii = np.arange(HP)
    bmask = np.zeros((NKPL, NIJ), np.float32)
    for kap in range(NKPL):
        di, dj = kap // 3, kap % 3
        rin = (ii + di >= 1) & (ii + di <= 14)
        cin_ = (ii + dj >= 1) & (ii + dj <= 14)
        bmask[kap] = np.outer(rin, cin_).reshape(-1)
    bmask = bmask.astype(ml_dtypes.bfloat16)

    w1T = np.ascontiguousarray(
        w1.transpose(1, 2, 3, 0).reshape(INTER, 9, INTER)
    ).astype(ml_dtypes.bfloat16)
    w2T = np.ascontiguousarray(
        w2.transpose(1, 2, 3, 0).reshape(INTER, 9, INTER)
    ).astype(ml_dtypes.bfloat16)

    # wfT packed [row 0..127, (ct, kt-group, kpl, p)]: contraction rows
    # grouped in 5 chunks of 128 (chunk 4 zero-padded 64..127)
    wf5 = wf.reshape(CH, NKPL, INTER, 3, 3)
    wf5 = wf5.transpose(3, 4, 2, 1, 0)          # (kh, kw, cin, kpl, c)
    wfTp = np.zeros((576, NKPL, CHP), np.float32)
    wfTp[:, :, padc] = wf5.reshape(576, NKPL, CH)
    wfTp = wfTp.reshape(576, NKPL, NCT, 128).transpose(0, 2, 1, 3)
    wfG = np.zeros((5, 128, NCT, NKPL, 128), np.float32)
    wfR = wfTp.reshape(576, NCT, NKPL, 128)
    for g in range(5):
        rows = wfR[128 * g:128 * (g + 1)]
        wfG[g, :rows.shape[0]] = rows
    wfT = np.ascontiguousarray(
        wfG.transpose(1, 2, 0, 3, 4).reshape(128, NCT * 5 * NKPL * 128)
    ).astype(ml_dtypes.bfloat16)

    woP = np.zeros((HIDP, DIM), np.float32)
    woP[:HID] = W_out.T
    woutT = np.ascontiguousarray(
        woP.reshape(NGT, 128, DIM).transpose(1, 0, 2).reshape(128, NGT * DIM)
    ).astype(ml_dtypes.bfloat16)

    return dict(winT=winT, w0S=w0S, bmask=bmask, w1T=w1T, w2T=w2T,
                wfT=wfT, woutT=woutT,
                identD=np.eye(128, dtype=ml_dtypes.bfloat16))


def kernel(**inputs) -> np.ndarray:
    if "nc" not in _CACHE:
        _CACHE["nc"] = _build_nc()
    nc = _CACHE["nc"]

    x = np.asarray(inputs["x"], np.float32)     # (64, 256, 14, 14)
    shared = _pack_shared(inputs)

    in_maps = []
    for c in range(NCORES):
        xc = x[BPC * c:BPC * (c + 1)]           # (8, 256, 14, 14)
        xT = np.ascontiguousarray(
            xc.transpose(1, 0, 2, 3).reshape(2, 128, BPC * NIJ)
            .transpose(1, 0, 2).reshape(128, 2 * BPC * NIJ)
        ).astype(ml_dtypes.bfloat16)
        m = dict(shared)
        m["xT"] = xT
        in_maps.append(m)

    res = run_bass_kernel_spmd(nc, in_maps, list(range(NCORES)))
    outs = []
    for c in range(NCORES):
        o = res.results[c]["out_f"].reshape(DIM, BPC, HP, HP)
        outs.append(o.transpose(1, 0, 2, 3))
    return np.ascontiguousarray(np.concatenate(outs, axis=0), dtype=np.float32)


# revision 34
# speedup vs baseline: 1.0231x; 1.0231x over previous
"""FRFN forward kernel for 8 Trainium2 NeuronCores.

Sharding: pure data parallel over batch B=64 -> 8 batches per core.
The TVConv generated weight (batch-independent) is recomputed on every
core.

Per-core pipeline (channel dims padded so x1/x2 halves align at 768):
  weightgen: 3x (3x3 conv + LayerNorm(CHW) + relu) on 4x14x14 posi map
             then final conv -> wgt[ct] = (128, 9, 196) bf16 per tile
  proj_in  : h = W_in @ x        PE bf16 -> padded (128, 8, 16, 16)
  tvconv   : prod_k = wgt_k * h_win_k   DVE (6 taps) + Pool (3 taps)
             partial pre-adds on DVE, remaining streams summed on the
             PE as identity-matmul PSUM accumulation
  gate     : x1 tiles: gelu straight from PSUM (ACT)
             x2 tiles: gated = ga * psum  (DVE, mixed dtype)
  proj_out : W_out @ gated               PE bf16
"""

import numpy as np
import ml_dtypes
from contextlib import ExitStack

import concourse.bacc as bacc
import concourse.bass as bass
import concourse.mybir as mybir
import concourse.tile as tile
from concourse.bass_utils import run_bass_kernel_spmd

F32 = mybir.dt.float32
BF16 = mybir.dt.bfloat16
AF = mybir.ActivationFunctionType
OP = mybir.AluOpType

NCORES = 8
B = 64
BPC = B // NCORES          # 8 batches per core
DIM = 256
HID = 680
CH = 2 * HID               # 1360
HIDP = 768                 # padded x1/x2 half (6 * 128)
CHP = 2 * HIDP             # 1536
NCT = CHP // 128           # 12 channel tiles
NGT = HIDP // 128          # 6 gate tiles
HP = 14
NIJ = HP * HP              # 196
PH = 16                    # padded spatial side
INTER = 64
NKPL = 9                   # 3x3 taps
KT_ROWS = [128, 128, 128, 128, 64]   # 576 contraction rows
NCHUNK = 4                 # 392-col psum chunks (2 batches x 196)
NB2 = 2 * NIJ              # 392
EPS = 1e-5
NLN = float(INTER * NIJ)

# channel-tile visit order: gate pairs adjacent (x1 then its x2)
CT_ORDER = [0, 6, 1, 7, 2, 8, 3, 9, 4, 10, 5, 11]
# taps 7,8 on the Pool engine; 0-6 on DVE. Pre-adds mostly DVE, with
# one per x2-stage on Pool (scalar_tensor_tensor, cheaper than Pool TT).
# Streams ordered by readiness; the last is closed early NEXT stage.
POOL_TAPS = (7, 8)
PRE_A3x = [(0, 1), (2, 3), (4, 5)]            # x1: all on DVE
STREAMS3 = [7, 8, 6, 0, 2, 4]                 # close: q45
PRE_A4x = [(0, 1), (2, 3), (6, 7), (4, 5)]    # x2: (4,5) rides on Pool
STREAMS4 = [8, 0, 2, 4, 6]                    # close: q67

_CACHE = {}


def _build_nc():
    nc = bacc.Bacc("TRN2", target_bir_lowering=False)

    xT = nc.dram_tensor("xT", [128, 2 * BPC * NIJ], BF16,
                        kind="ExternalInput")
    winT = nc.dram_tensor("winT", [128, 2 * CHP], BF16,
                          kind="ExternalInput")
    # first conv folded host-side: posi_map is all-ones, so conv0 reduces to
    # (sum_cin w0)[tap, c] @ border_mask[tap, ij]
    w0S = nc.dram_tensor("w0S", [NKPL, INTER], BF16, kind="ExternalInput")
    bmask = nc.dram_tensor("bmask", [NKPL, NIJ], BF16, kind="ExternalInput")
    w1T = nc.dram_tensor("w1T", [INTER, NKPL, INTER], BF16,
                         kind="ExternalInput")
    w2T = nc.dram_tensor("w2T", [INTER, NKPL, INTER], BF16,
                         kind="ExternalInput")
    wfT = nc.dram_tensor("wfT", [128, NCT * 5 * NKPL * 128], BF16,
                         kind="ExternalInput")
    woutT = nc.dram_tensor("woutT", [128, NGT * DIM], BF16,
                           kind="ExternalInput")
    identD = nc.dram_tensor("identD", [128, 128], BF16, kind="ExternalInput")
    out_f = nc.dram_tensor("out_f", [DIM, BPC * NIJ], F32,
                           kind="ExternalOutput")

    with tile.TileContext(nc) as tc, ExitStack() as ctx:
        persist = ctx.enter_context(tc.tile_pool(name="persist", bufs=1))
        work = ctx.enter_context(tc.tile_pool(name="work", bufs=2))
        wgtpool = ctx.enter_context(tc.tile_pool(name="wgtpool", bufs=2))
        prodpool = ctx.enter_context(tc.tile_pool(name="prodpool", bufs=1))
        gapool = ctx.enter_context(tc.tile_pool(name="gapool", bufs=2))
        wfpool = ctx.enter_context(tc.tile_pool(name="wfpool", bufs=2))
        # 4 rolling single-bank psum units + one 4-bank accumulator
        ps_u = ctx.enter_context(
            tc.tile_pool(name="ps_u", bufs=4, space="PSUM"))
        ps_pst = ctx.enter_context(
            tc.tile_pool(name="ps_pst", bufs=1, space="PSUM"))

        # ---------------- persistent SBUF tensors ----------------
        h_sb = [persist.tile([128, BPC, PH, PH], BF16, name="t", tag=f"h{i}")
                for i in range(NCT)]
        gated = [persist.tile([128, BPC * NIJ], BF16, name="t", tag=f"gd{i}")
                 for i in range(NGT)]
        wout_sb = persist.tile([128, NGT, DIM], BF16, name="t", tag="wo")
        x_sb = persist.tile([128, 2, BPC * NIJ], BF16, name="t", tag="x")
        win_sb = persist.tile([128, 2, CHP], BF16, name="t", tag="wi")

        w0_sb = persist.tile([NKPL, INTER], BF16, name="t", tag="w0")
        bm_sb = persist.tile([NKPL, NIJ], BF16, name="t", tag="bm")
        w1_sb = persist.tile([INTER, NKPL, INTER], BF16, name="t", tag="w1")
        w2_sb = persist.tile([INTER, NKPL, INTER], BF16, name="t", tag="w2")
        pad1 = persist.tile([INTER, PH, PH], BF16, name="t", tag="pad1")
        pad2 = persist.tile([INTER, PH, PH], BF16, name="t", tag="pad2")
        pad3 = persist.tile([INTER, PH, PH], BF16, name="t", tag="pad3")
        p3 = [persist.tile([128, NIJ], BF16, name="t", tag=f"p3_{k}")
              for k in range(5)]
        ones_b = persist.tile([INTER, INTER], F32, name="t", tag="ones_b")
        ident = persist.tile([128, 128], BF16, name="t", tag="ident")
        eps_v = persist.tile([INTER, 1], F32, name="t", tag="eps")

        # ---------------- input DMAs + memsets ----------------
        # tiny LN-chain inputs first (unblock the serial head chain), then
        # ident, x/win; wf streams next (emitted below); wout last (tail)
        nc.sync.dma_start(w0_sb[:], w0S[:])
        nc.sync.dma_start(bm_sb[:], bmask[:])
        nc.sync.dma_start(w1_sb[:], w1T[:])
        nc.sync.dma_start(w2_sb[:], w2T[:])
        nc.sync.dma_start(ident[:], identD[:])
        nc.sync.dma_start(x_sb[:].rearrange("p a b -> p (a b)"), xT[:])
        nc.sync.dma_start(win_sb[:].rearrange("p a b -> p (a b)"), winT[:])
        nc.sync.dma_start(wout_sb[:].rearrange("p a b -> p (a b)"), woutT[:])

        # pre-warm the sqrt ACT table (LN chain); the gelu table is warmed
        # by a dummy activation right after the chain so the 1.3us table
        # load lands off the critical path
        warm = persist.tile([1, 1], F32, name="t", tag="warm")
        wsink = persist.tile([1, 1], F32, name="t", tag="wsink")
        nc.gpsimd.memset(warm[:], 1.0)
        nc.scalar.activation(wsink[:], warm[:], AF.Sqrt)

        nc.gpsimd.memset(ones_b[:], 1.0)
        nc.gpsimd.memset(eps_v[:], EPS)
        nc.gpsimd.memset(pad1[:], 0.0)
        nc.gpsimd.memset(pad2[:], 0.0)
        nc.gpsimd.memset(pad3[:], 0.0)
        nc.vector.memset(p3[4][64:128, :], 0.0)
        for i in CT_ORDER:
            # zero only the pad borders (proj_in drains fill the interior);
            # on DVE, idle during the head
            t = h_sb[i]
            nc.vector.memset(t[:, :, 0, :], 0.0)
            nc.vector.memset(t[:, :, 15, :], 0.0)
            nc.vector.memset(t[:, :, 1:15, 0], 0.0)
            nc.vector.memset(t[:, :, 1:15, 15], 0.0)

        # ------------- weight-gen small conv chain (fp32) -------------
        # LayerNorm([C,H,W]) with the reference's g=1, b=0 folded out.
        # Stats are reduced AND broadcast in one matmul with an all-ones
        # [64,64] stationary, so every partition holds [sum, sumsq] and the
        # normalize reads the conv result straight from PSUM.
        def layernorm_relu(ps_in, pad_tile):
            sq = work.tile([INTER, NIJ], F32, name="t", tag="ln_sq")
            stats = work.tile([INTER, 2], F32, name="t", tag="ln_st")
            nc.scalar.activation(sq[:], ps_in, AF.Square,
                                 accum_out=stats[:, 1:2])
            nc.vector.tensor_reduce(stats[:, 0:1], ps_in,
                                    mybir.AxisListType.X, OP.add)
            ps_bc = ps_u.tile([INTER, 2], F32, name="t", tag="u")
            nc.tensor.matmul(ps_bc[:], ones_b[:], stats[:],
                             start=True, stop=True)
            mr2 = work.tile([INTER, 2], F32, name="t", tag="ln_mr")
            nc.vector.tensor_scalar_mul(mr2[:], ps_bc[:], 1.0 / NLN)
            musq = work.tile([INTER, 1], F32, name="t", tag="ln_musq")
            nc.vector.tensor_mul(musq[:], mr2[:, 0:1], mr2[:, 0:1])
            var = work.tile([INTER, 1], F32, name="t", tag="ln_var")
            nc.vector.tensor_sub(var[:], mr2[:, 1:2], musq[:])
            std = work.tile([INTER, 1], F32, name="t", tag="ln_std")
            nc.scalar.activation(std[:], var[:], AF.Sqrt, bias=eps_v[:])
            rstd = work.tile([INTER, 1], F32, name="t", tag="ln_rstd")
            nc.vector.reciprocal(rstd[:], std[:])
            xn = work.tile([INTER, NIJ], F32, name="t", tag="ln_xn")
            nc.vector.tensor_scalar(xn[:], ps_in, mr2[:, 0:1], rstd[:],
                                    op0=OP.subtract, op1=OP.mult)
            dst = pad_tile[:, 1:15, 1:15]
            src = xn[:].rearrange("p (i j) -> p i j", i=HP, j=HP)
            nc.scalar.activation(dst, src, AF.Relu)

        def conv3x3(w_sb, pad_tile, ps_out):
            for kap in range(NKPL):
                di, dj = kap // 3, kap % 3
                nc.tensor.matmul(ps_out, w_sb[:, kap, :],
                                 pad_tile[:, di:di + HP, dj:dj + HP],
                                 start=(kap == 0), stop=(kap == NKPL - 1))

        # ---------------- per-stage emission helpers ----------------
        def emit_proj_in(ct):
            """proj_in for one channel tile -> h_sb[ct] (padded layout)."""
            for ch in range(NCHUNK):
                u = ps_u.tile([128, NB2], F32, name="t", tag="u")
                for kt in range(2):
                    nc.tensor.matmul(
                        u[:],
                        win_sb[:, kt, 128 * ct:128 * (ct + 1)],
                        x_sb[:, kt, NB2 * ch:NB2 * (ch + 1)],
                        start=(kt == 0), stop=(kt == 1))
                dst = h_sb[ct][:, 2 * ch:2 * ch + 2, 1:15, 1:15]
                src = u[:].rearrange("p (b i j) -> p b i j", b=2, i=HP, j=HP)
                nc.scalar.activation(dst, src, AF.Copy)

        def emit_wf_load(ct):
            t = wfpool.tile([128, 5, NKPL * 128], BF16, name="t", tag="wf")
            c0 = 5 * NKPL * 128 * ct
            nc.sync.dma_start(t[:].rearrange("p a b -> p (a b)"),
                              wfT[:, c0:c0 + 5 * NKPL * 128])
            return t

        def emit_conv_f(ct, wf_t):
            """final conv for one channel tile -> wgt (128, 9, 196) bf16.
            Two taps share one psum bank; drained in 2-tap batches."""
            wgt = wgtpool.tile([128, NKPL, NIJ], BF16, name="t", tag="wgt")
            for t0 in range(0, NKPL, 2):
                ntap = min(2, NKPL - t0)
                u = ps_u.tile([128, NB2], F32, name="t", tag="u")
                for sub in range(ntap):
                    kpl = t0 + sub
                    dst = u[:, NIJ * sub:NIJ * (sub + 1)]
                    for kt in range(5):
                        nc.tensor.matmul(
                            dst,
                            wf_t[:, kt, 128 * kpl:128 * (kpl + 1)],
                            p3[kt][:],
                            start=(kt == 0), stop=(kt == 4))
                nc.scalar.activation(
                    wgt[:, t0:t0 + ntap, :],
                    u[:, 0:NIJ * ntap].rearrange("p (t f) -> p t f", t=ntap),
                    AF.Copy)
            return wgt

        def emit_mult(ct, wgt, t, eng):
            di, dj = t // 3, t % 3
            wgb = (wgt[:, t, :].rearrange("p (i j) -> p i j", i=HP, j=HP)
                   .unsqueeze(1).broadcast_to((128, BPC, HP, HP)))
            hwin = h_sb[ct][:, :, di:di + HP, dj:dj + HP]
            prod = prodpool.tile([128, BPC * NIJ], BF16,
                                 name="t", tag=f"prod{t}")
            pr = prod[:].rearrange("p (b i j) -> p b i j",
                                   b=BPC, i=HP, j=HP)
            eng.tensor_mul(pr, hwin, wgb)
            return prod

        def emit_identity(pst, prod, start, stop):
            for ch in range(NCHUNK):
                nc.tensor.matmul(
                    pst[:, ch, 0:NB2], ident[:],
                    prod[:, NB2 * ch:NB2 * (ch + 1)],
                    start=start, stop=stop)

        def emit_gate(pend):
            """Chunked gate for a finished tile: frees pst incrementally."""
            pst_p, ct_p = pend
            if ct_p < NGT:
                # x1 tile: gelu straight from psum, per 392-col chunk
                ga = gapool.tile([128, BPC * NIJ], BF16, name="t",
                                 tag=f"ga{ct_p}")
                for c in range(NCHUNK):
                    nc.scalar.activation(
                        ga[:, NB2 * c:NB2 * (c + 1)],
                        pst_p[:, c, 0:NB2], AF.Gelu)
                ga_map[ct_p] = ga
            else:
                # x2 tile: gated = gelu(x1) * x2 on Pool (one 3D STT op)
                ga = ga_map[ct_p - NGT]
                nc.gpsimd.scalar_tensor_tensor(
                    gated[ct_p - NGT][:].rearrange(
                        "p (c f) -> p c f", c=NCHUNK),
                    pst_p[:, :, 0:NB2], 1.0,
                    ga[:].rearrange("p (c f) -> p c f", c=NCHUNK),
                    op0=OP.mult, op1=OP.mult)

        # =================== program ===================
        # start streaming the first two tiles' conv-f weights right away
        wf0 = emit_wf_load(CT_ORDER[0])
        wf1 = emit_wf_load(CT_ORDER[1])

        ps0 = ps_u.tile([INTER, NIJ], F32, name="t", tag="u")
        # conv0 folded to a single matmul (posi_map == ones)
        nc.tensor.matmul(ps0[:], w0_sb[:], bm_sb[:], start=True, stop=True)
        layernorm_relu(ps0[:], pad1)

        # proj_in prologue interleaved with the LN chain
        emit_proj_in(CT_ORDER[0])

        ps1 = ps_u.tile([INTER, NIJ], F32, name="t", tag="u")
        conv3x3(w1_sb, pad1, ps1[:])
        layernorm_relu(ps1[:], pad2)

        emit_proj_in(CT_ORDER[1])

        ps2 = ps_u.tile([INTER, NIJ], F32, name="t", tag="u")
        conv3x3(w2_sb, pad2, ps2[:])
        layernorm_relu(ps2[:], pad3)

        # warm the gelu table now (off the critical path)
        nc.scalar.activation(wsink[:], warm[:], AF.Gelu)

        # im2col of pad3 for the final conv (576 contraction rows)
        qengs = [nc.sync, nc.scalar]
        for kt in range(5):
            nk = KT_ROWS[kt] // 64
            for sub in range(nk):
                kap = 2 * kt + sub
                di, dj = kap // 3, kap % 3
                srcw = pad3[:, di:di + HP, dj:dj + HP]
                dst = p3[kt][64 * sub:64 * (sub + 1), :]
                dst = dst.rearrange("p (i j) -> p i j", i=HP, j=HP)
                qengs[kap % 2].dma_start(dst, srcw)

        # conv-f runs one stage ahead of the tap loop so wgt is drained
        # before the stage's first DVE mult
        wgt_next = emit_conv_f(CT_ORDER[0], wf0)
        wf_next = wf1
        ga_map = {}
        pending_close = None   # (pst, last_prod, ct)
        for s, ct in enumerate(CT_ORDER):
            wgt = wgt_next

            # close the previous tile's identity group first thing (its last
            # product finished at the end of the previous stage)
            gate_pend = None
            if pending_close is not None:
                pst_p, last_p, ct_p = pending_close
                emit_identity(pst_p, last_p, start=False, stop=True)
                gate_pend = (pst_p, ct_p)
                pending_close = None

            if s + 1 < NCT:
                wgt_next = emit_conv_f(CT_ORDER[s + 1], wf_next)
            if s + 2 < NCT:
                wf_next = emit_wf_load(CT_ORDER[s + 2])
            if s + 3 < NCT:
                emit_proj_in(CT_ORDER[s + 3])

            # Pool: tap7, [x1: gate for the previous x2 tile], tap8;
            # DVE: taps 0-6 then pre-adds. On x2 stages the (4,5) pre-add
            # rides on Pool via scalar_tensor_tensor.
            x1 = s % 2 == 0
            prods = {}
            prods[7] = emit_mult(ct, wgt, 7, nc.gpsimd)
            if x1 and gate_pend is not None:
                emit_gate(gate_pend)          # Pool STT, ~2.3us
                gate_pend = None
            prods[8] = emit_mult(ct, wgt, 8, nc.gpsimd)
            for t in (0, 1, 2, 3):
                prods[t] = emit_mult(ct, wgt, t, nc.vector)
            if gate_pend is not None:
                emit_gate(gate_pend)          # x1-gelu on ACT
            for t in (4, 5, 6):
                prods[t] = emit_mult(ct, wgt, t, nc.vector)

            preadds = PRE_A3x if x1 else PRE_A4x
            merged = set()
            for (a, b) in preadds:
                if not x1 and (a, b) == (4, 5):
                    nc.gpsimd.scalar_tensor_tensor(
                        prods[a][:], prods[a][:], 1.0, prods[b][:],
                        op0=OP.mult, op1=OP.add)
                else:
                    nc.vector.tensor_add(prods[a][:], prods[a][:],
                                         prods[b][:])
                merged.add(b)
            streams = [prods[t] for t in (STREAMS3 if x1 else STREAMS4)
                       if t not in merged]

            pst = ps_pst.tile([128, NCHUNK, 512], F32, name="t", tag="pst")
            for si, prod in enumerate(streams[:-1]):
                emit_identity(pst, prod, start=(si == 0), stop=False)
            pending_close = (pst, streams[-1], ct)

        # epilogue: close and gate the last tile
        pst_p, last_p, ct_p = pending_close
        emit_identity(pst_p, last_p, start=False, stop=True)
        emit_gate((pst_p, ct_p))

        # ---------------- proj_out: W_out @ gated ----------------
        outpool = ctx.enter_context(tc.tile_pool(name="outpool", bufs=4))
        for m in range(2):
            for ch in range(NCHUNK):
                u = ps_u.tile([128, NB2], F32, name="t", tag="u")
                for kt in range(NGT):
                    nc.tensor.matmul(
                        u[:],
                        wout_sb[:, kt, 128 * m:128 * (m + 1)],
                        gated[kt][:, NB2 * ch:NB2 * (ch + 1)],
                        start=(kt == 0), stop=(kt == NGT - 1))
                ot = outpool.tile([128, NB2], F32, name="t", tag="ot")
                nc.scalar.activation(ot[:], u[:], AF.Copy)
                nc.sync.dma_start(
                    out_f[128 * m:128 * (m + 1), NB2 * ch:NB2 * (ch + 1)],
                    ot[:])

    nc.compile()
    return nc


def _pack_shared(inputs):
    """Pack the batch-independent tensors (host-side layout marshalling)."""
    W_in = np.asarray(inputs["W_in"], np.float32)
    W_out = np.asarray(inputs["W_out"], np.float32)
    w0 = np.asarray(inputs["w0"], np.float32)
    w1 = np.asarray(inputs["w1"], np.float32)
    w2 = np.asarray(inputs["w2"], np.float32)
    wf = np.asarray(inputs["wf"], np.float32)

    padc = np.arange(CH)
    padc = np.where(padc < HID, padc, padc + (HIDP - HID))

    winP = np.zeros((CHP, DIM), np.float32)
    winP[padc] = W_in
    winT = np.ascontiguousarray(
        winP.T.reshape(2, 128, CHP).transpose(1, 0, 2).reshape(128, 2 * CHP)
    ).astype(ml_dtypes.bfloat16)

    # conv0 on the all-ones posi_map folds to (sum_cin w0) x border-mask
    w0S = np.ascontiguousarray(
        w0.sum(axis=1).reshape(INTER, 9).T).astype(ml_dtypes.bfloat16)
    ii = np.arange(HP)
    bmask = np.zeros((NKPL, NIJ), np.float32)
    for kap in range(NKPL):
        di, dj = kap // 3, kap % 3
        rin = (ii + di >= 1) & (ii + di <= 14)
        cin_ = (ii + dj >= 1) & (ii + dj <= 14)
        bmask[kap] = np.outer(rin, cin_).reshape(-1)
    bmask = bmask.astype(ml_dtypes.bfloat16)

    w1T = np.ascontiguousarray(
        w1.transpose(1, 2, 3, 0).reshape(INTER, 9, INTER)
    ).astype(ml_dtypes.bfloat16)
    w2T = np.ascontiguousarray(
        w2.transpose(1, 2, 3, 0).reshape(INTER, 9, INTER)
    ).astype(ml_dtypes.bfloat16)

    # wfT packed [row 0..127, (ct, kt-group, kpl, p)]: contraction rows
    # grouped in 5 chunks of 128 (chunk 4 zero-padded 64..127)
    wf5 = wf.reshape(CH, NKPL, INTER, 3, 3)
    wf5 = wf5.transpose(3, 4, 2, 1, 0)          # (kh, kw, cin, kpl, c)
    wfTp = np.zeros((576, NKPL, CHP), np.float32)
    wfTp[:, :, padc] = wf5.reshape(576, NKPL, CH)
    wfTp = wfTp.reshape(576, NKPL, NCT, 128).transpose(0, 2, 1, 3)
    wfG = np.zeros((5, 128, NCT, NKPL, 128), np.float32)
    wfR = wfTp.reshape(576, NCT, NKPL, 128)
    for g in range(5):
        rows = wfR[128 * g:128 * (g + 1)]
        wfG[g, :rows.shape[0]] = rows
    wfT = np.ascontiguousarray(
        wfG.transpose(1, 2, 0, 3, 4).reshape(128, NCT * 5 * NKPL * 128)
    ).astype(ml_dtypes.bfloat16)

    woP = np.zeros((HIDP, DIM), np.float32)
    woP[:HID] = W_out.T
    woutT = np.ascontiguousarray(
        woP.reshape(NGT, 128, DIM).transpose(1, 0, 2).reshape(128, NGT * DIM)
    ).astype(ml_dtypes.bfloat16)

    return dict(winT=winT, w0S=w0S, bmask=bmask, w1T=w1T, w2T=w2T,
                wfT=wfT, woutT=woutT,
                identD=np.eye(128, dtype=ml_dtypes.bfloat16))


def kernel(**inputs) -> np.ndarray:
    if "nc" not in _CACHE:
        _CACHE["nc"] = _build_nc()
    nc = _CACHE["nc"]

    x = np.asarray(inputs["x"], np.float32)     # (64, 256, 14, 14)
    shared = _pack_shared(inputs)

    in_maps = []
    for c in range(NCORES):
        xc = x[BPC * c:BPC * (c + 1)]           # (8, 256, 14, 14)
        xT = np.ascontiguousarray(
            xc.transpose(1, 0, 2, 3).reshape(2, 128, BPC * NIJ)
            .transpose(1, 0, 2).reshape(128, 2 * BPC * NIJ)
        ).astype(ml_dtypes.bfloat16)
        m = dict(shared)
        m["xT"] = xT
        in_maps.append(m)

    res = run_bass_kernel_spmd(nc, in_maps, list(range(NCORES)))
    outs = []
    for c in range(NCORES):
        o = res.results[c]["out_f"].reshape(DIM, BPC, HP, HP)
        outs.append(o.transpose(1, 0, 2, 3))
    return np.ascontiguousarray(np.concatenate(outs, axis=0), dtype=np.float32)


# revision 35
# speedup vs baseline: 1.0866x; 1.0621x over previous
"""FRFN forward kernel for 8 Trainium2 NeuronCores.

Sharding: pure data parallel over batch B=64 -> 8 batches per core.
The TVConv generated weight (batch-independent) is recomputed on every
core (its cost hides under the DVE-bound tvconv stage).

Per-core pipeline (all channel dims padded so x1/x2 halves align):
  proj_in  : h = W_in @ x          PE bf16, output bf16 in padded
                                   (128, 8, 16, 16) spatial layout
  weightgen: 3x (3x3 conv + LayerNorm(CHW) + relu) on 4x14x14 posi map
             (fp32, tiny) then final conv -> wgt (CH*9, 196)  PE bf16
  tvconv   : out[c,b,ij] = sum_k wgt[c,k,ij] * h[c,b,ij@k]    DVE+Pool
  gate     : gelu(x1) * x2                                    ACT+Pool
  proj_out : W_out @ gated                                    PE bf16
"""

import numpy as np
import ml_dtypes
from contextlib import ExitStack

import concourse.bacc as bacc
import concourse.bass as bass
import concourse.mybir as mybir
import concourse.tile as tile
from concourse.bass_utils import run_bass_kernel_spmd

F32 = mybir.dt.float32
BF16 = mybir.dt.bfloat16
AF = mybir.ActivationFunctionType
OP = mybir.AluOpType

NCORES = 8
B = 64
BPC = B // NCORES          # 8 batches per core
DIM = 256
HID = 680
CH = 2 * HID               # 1360
HIDP = 768                 # padded x1/x2 half (6 * 128)
CHP = 2 * HIDP             # 1536
NCT = CHP // 128           # 12 channel tiles
NGT = HIDP // 128          # 6 gate tiles
HP = 14
NIJ = HP * HP              # 196
PH = 16                    # padded spatial side
PHW = PH * PH              # 256
INTER = 64
NKPL = 9                   # 3x3 taps
KT_ROWS = [128, 128, 128, 128, 64]   # 576 contraction rows for 3x3xINTER convs
NCHUNK = 4                 # N chunks for the big matmuls (2 batches x 196)
NB2 = 2 * NIJ              # 392
EPS = 1e-5
NLN = float(INTER * NIJ)   # layernorm normalizes over (C,H,W) = 64*196

# tap mults offloaded from DVE to the otherwise-idle Pool engine; their
# wgt is ready early in the per-tile stream and Pool has ~6.4us of
# serial time for the two of them inside an ~11us tile stage
POOL_TAPS = (0, 5)

_CACHE = {}


def _build_nc(reps=1):
    nc = bacc.Bacc("TRN2", target_bir_lowering=False)

    xT = nc.dram_tensor("xT", [DIM, BPC * NIJ], BF16, kind="ExternalInput")
    winT = nc.dram_tensor("winT", [DIM, CHP], BF16, kind="ExternalInput")
    posiP = nc.dram_tensor("posiP", [4, PH, PH], BF16, kind="ExternalInput")
    w0T = nc.dram_tensor("w0T", [4, NKPL, INTER], BF16, kind="ExternalInput")
    w1T = nc.dram_tensor("w1T", [INTER, NKPL, INTER], BF16,
                         kind="ExternalInput")
    w2T = nc.dram_tensor("w2T", [INTER, NKPL, INTER], BF16,
                         kind="ExternalInput")
    gb = nc.dram_tensor("gb", [INTER, 6, NIJ], F32, kind="ExternalInput")
    wfT = nc.dram_tensor("wfT", [576, NKPL * CHP], BF16, kind="ExternalInput")
    woutT = nc.dram_tensor("woutT", [HIDP, DIM], BF16, kind="ExternalInput")
    identD = nc.dram_tensor("identD", [128, 128], BF16, kind="ExternalInput")
    out_f = nc.dram_tensor("out_f", [DIM, BPC * NIJ], F32, kind="ExternalOutput")

    with tile.TileContext(nc) as tc, ExitStack() as ctx:
        persist = ctx.enter_context(tc.tile_pool(name="persist", bufs=1))
        work = ctx.enter_context(tc.tile_pool(name="work", bufs=2))
        wgtpool = ctx.enter_context(tc.tile_pool(name="wgtpool", bufs=9))
        prodpool = ctx.enter_context(tc.tile_pool(name="prodpool", bufs=1))
        outpool = ctx.enter_context(tc.tile_pool(name="outpool", bufs=4))
        wfpool2 = ctx.enter_context(tc.tile_pool(name="wfpool2", bufs=2))
        ps_proj = ctx.enter_context(
            tc.tile_pool(name="ps_proj", bufs=2, space="PSUM"))
        ps_f = ctx.enter_context(
            tc.tile_pool(name="ps_f", bufs=2, space="PSUM"))
        ps_tv = ctx.enter_context(
            tc.tile_pool(name="ps_tv", bufs=1, space="PSUM"))
        ps_s = ps_f

        # ---------------- persistent SBUF tensors ----------------
        h_sb = [persist.tile([128, BPC, PH, PH], BF16, name="t", tag=f"h{i}")
                for i in range(NCT)]
        tvacc = [persist.tile([128, BPC * NIJ], BF16, name="t", tag=f"tv{i}")
                 for i in range(NCT)]
        wout_sb = [persist.tile([128, DIM], BF16, name="t", tag=f"wo{i}")
                   for i in range(NGT)]

        # small-conv chain buffers
        posi_sb = persist.tile([4, PH, PH], BF16, name="t", tag="posi")
        w0_sb = persist.tile([4, NKPL, INTER], BF16, name="t", tag="w0")
        w1_sb = persist.tile([INTER, NKPL, INTER], BF16, name="t", tag="w1")
        w2_sb = persist.tile([INTER, NKPL, INTER], BF16, name="t", tag="w2")
        gb_sb = persist.tile([INTER, 6, NIJ], F32, name="t", tag="gb")
        pad1 = persist.tile([INTER, PH, PH], BF16, name="t", tag="pad1")
        pad2 = persist.tile([INTER, PH, PH], BF16, name="t", tag="pad2")
        pad3 = persist.tile([INTER, PH, PH], BF16, name="t", tag="pad3")
        p3 = [persist.tile([KT_ROWS[k], NIJ], BF16, name="t", tag=f"p3_{k}")
              for k in range(5)]
        ones_c = persist.tile([INTER, 1], F32, name="t", tag="ones_c")
        ones_r = persist.tile([1, INTER], F32, name="t", tag="ones_r")
        ident = persist.tile([128, 128], BF16, name="t", tag="ident")
        eps_t = persist.tile([1, 1], F32, name="t", tag="eps")

        # ---------------- input DMAs + memsets ----------------
        nc.sync.dma_start(posi_sb[:], posiP[:])
        nc.sync.dma_start(ident[:], identD[:])
        nc.sync.dma_start(w0_sb[:], w0T[:])
        nc.sync.dma_start(w1_sb[:], w1T[:])
        nc.sync.dma_start(w2_sb[:], w2T[:])
        nc.sync.dma_start(gb_sb[:], gb[:])
        for i in range(NGT):
            nc.sync.dma_start(wout_sb[i][:], woutT[128 * i:128 * (i + 1), :])

        # pre-warm the ACT function tables off the critical path (table
        # switches cost ~1.3us each and would otherwise fire mid-LN-chain)
        warm = persist.tile([1, 1], F32, name="t", tag="warm")
        nc.gpsimd.memset(warm[:], 1.0)
        wsink = persist.tile([1, 1], F32, name="t", tag="wsink")
        for fn in (AF.Sqrt, AF.Relu, AF.Gelu, AF.Identity):
            nc.scalar.activation(wsink[:], warm[:], fn)

        nc.gpsimd.memset(ones_c[:], 1.0)
        nc.gpsimd.memset(eps_t[:], EPS)
        nc.gpsimd.memset(ones_r[:], 1.0)
        nc.gpsimd.memset(pad1[:], 0.0)
        nc.gpsimd.memset(pad2[:], 0.0)
        nc.gpsimd.memset(pad3[:], 0.0)
        for i in range(NCT):
            # zero only the pad borders (the 14x14 interior gets overwritten
            # by the proj_in drains)
            t = h_sb[i]
            nc.gpsimd.memset(t[:, :, 0, :], 0.0)
            nc.gpsimd.memset(t[:, :, 15, :], 0.0)
            nc.gpsimd.memset(t[:, :, 1:15, 0], 0.0)
            nc.gpsimd.memset(t[:, :, 1:15, 15], 0.0)

        def emit_body():
          # ------------- weight-gen small conv chain (fp32) -------------
          def layernorm_relu(ps_in, g_ap, b_ap, pad_tile):
              """ps_in: PSUM (64,196) conv output. Writes relu(LN(x)*g+b) into
              pad_tile[:, 1:15, 1:15] (borders stay zero)."""
              sq = work.tile([INTER, NIJ], F32, name="t", tag="ln_sq")
              hval = work.tile([INTER, NIJ], F32, name="t", tag="ln_h")
              stats = work.tile([INTER, 2], F32, name="t", tag="ln_st")
              nc.scalar.activation(sq[:], ps_in[:], AF.Square,
                                   accum_out=stats[:, 1:2])
              nc.scalar.activation(hval[:], ps_in[:], AF.Copy,
                                   accum_out=stats[:, 0:1])
              # cross-partition reduce: [sum; sumsq] = ones.T @ stats
              ps_r = ps_s.tile([1, 2], F32, name="t", tag="fc")
              nc.tensor.matmul(ps_r[:], ones_c[:], stats[:],
                               start=True, stop=True)
              mr = work.tile([1, 2], F32, name="t", tag="ln_mr")
              musq = work.tile([1, 1], F32, name="t", tag="ln_musq")
              nc.scalar.activation(musq[:], ps_r[:, 0:1], AF.Square,
                                   scale=1.0 / NLN)
              e2e = work.tile([1, 1], F32, name="t", tag="ln_e2e")
              nc.scalar.activation(e2e[:], ps_r[:, 1:2], AF.Identity,
                                   scale=1.0 / NLN, bias=eps_t[:])
              nc.scalar.activation(mr[:, 0:1], ps_r[:, 0:1], AF.Copy,
                                   scale=1.0 / NLN)
              std = work.tile([1, 1], F32, name="t", tag="ln_std")
              nc.scalar.activation(std[:], musq[:], AF.Sqrt,
                                   scale=-1.0, bias=e2e[:])
              nc.vector.reciprocal(mr[:, 1:2], std[:])
              # broadcast [mu, rstd] to all 64 partitions via rank-1 matmul
              ps_bc = ps_s.tile([INTER, 2], F32, name="t", tag="fc")
              nc.tensor.matmul(ps_bc[:], ones_r[:], mr[:], start=True, stop=True)
              bc = work.tile([INTER, 2], F32, name="t", tag="ln_bc")
              nc.scalar.activation(bc[:], ps_bc[:], AF.Copy)
              xn = work.tile([INTER, NIJ], F32, name="t", tag="ln_xn")
              nc.vector.tensor_scalar(xn[:], hval[:], bc[:, 0:1], bc[:, 1:2],
                                      op0=OP.subtract, op1=OP.mult)
              t2 = work.tile([INTER, NIJ], F32, name="t", tag="ln_t2")
              nc.vector.tensor_mul(t2[:], xn[:], g_ap)
              t3 = work.tile([INTER, NIJ], F32, name="t", tag="ln_t3")
              nc.vector.tensor_add(t3[:], t2[:], b_ap)
              dst = pad_tile[:, 1:15, 1:15]
              src = t3[:].rearrange("p (i j) -> p i j", i=HP, j=HP)
              nc.scalar.activation(dst, src, AF.Relu)

          def conv3x3(w_sb, pad_tile, ps_out):
              """3x3 conv as 9 accumulating matmuls over shifted windows."""
              for kap in range(NKPL):
                  di, dj = kap // 3, kap % 3
                  nc.tensor.matmul(ps_out[:], w_sb[:, kap, :],
                                   pad_tile[:, di:di + HP, dj:dj + HP],
                                   start=(kap == 0), stop=(kap == NKPL - 1))

          ps0 = ps_s.tile([INTER, NIJ], F32, name="t", tag="fc")
          conv3x3(w0_sb, posi_sb, ps0)
          layernorm_relu(ps0, gb_sb[:, 0, :], gb_sb[:, 1, :], pad1)
          ps1 = ps_s.tile([INTER, NIJ], F32, name="t", tag="fc")
          conv3x3(w1_sb, pad1, ps1)
          layernorm_relu(ps1, gb_sb[:, 2, :], gb_sb[:, 3, :], pad2)
          ps2 = ps_s.tile([INTER, NIJ], F32, name="t", tag="fc")
          conv3x3(w2_sb, pad2, ps2)
          layernorm_relu(ps2, gb_sb[:, 4, :], gb_sb[:, 5, :], pad3)
          # build the 576-row im2col for the final conv, 2 parallel queues
          qengs = [nc.sync, nc.scalar]
          for kt in range(5):
              nk = KT_ROWS[kt] // 64
              for sub in range(nk):
                  kap = 2 * kt + sub
                  di, dj = kap // 3, kap % 3
                  srcw = pad3[:, di:di + HP, dj:dj + HP]
                  dst = p3[kt][64 * sub:64 * (sub + 1), :]
                  dst = dst.rearrange("p (i j) -> p i j", i=HP, j=HP)
                  qengs[kap % 2].dma_start(dst, srcw)

          # ------- fused per-channel-tile loop: proj_in -> convf -> tvconv ----
          # wfT is packed ct-major: column ct*1152 + kpl*128 + p.
          # Per channel tile: proj_in matmuls fill the padded h tile; then 9
          # taps of conv-f -> wgt -> DVE/Pool product; the 9-tap sum runs on
          # the PE as identity-matmul PSUM accumulation (exact bf16 identity,
          # fp32 accumulate). Tiles visited in gate-pair order so gelu*gate
          # can fire as soon as a pair completes.
          x_sb = [persist.tile([128, BPC * NIJ], BF16, name="t", tag=f"x{i}")
                  for i in range(2)]
          win_sb = [persist.tile([128, CHP], BF16, name="t", tag=f"wi{i}")
                    for i in range(2)]
          for i in range(2):
              nc.sync.dma_start(x_sb[i][:], xT[128 * i:128 * (i + 1), :])
              nc.sync.dma_start(win_sb[i][:], winT[128 * i:128 * (i + 1), :])

          CT_ORDER = [0, 6, 1, 7, 2, 8, 3, 9, 4, 10, 5, 11]
          for ct in CT_ORDER:
              # proj_in for this channel tile
              for ch in range(NCHUNK):
                  ps = ps_proj.tile([128, NB2], F32, name="t", tag="pj")
                  for kt in range(2):
                      nc.tensor.matmul(
                          ps[:],
                          win_sb[kt][:, 128 * ct:128 * (ct + 1)],
                          x_sb[kt][:, NB2 * ch:NB2 * (ch + 1)],
                          start=(kt == 0), stop=(kt == 1))
                  # drain into padded (b, 16, 16) layout as bf16
                  dst = h_sb[ct][:, 2 * ch:2 * ch + 2, 1:15, 1:15]
                  src = ps[:].rearrange("p (b i j) -> p b i j",
                                        b=2, i=HP, j=HP)
                  nc.scalar.activation(dst, src, AF.Copy)

              # stream this tile's final-conv weights
              wf_t = []
              r0 = 0
              c0 = NKPL * 128 * ct
              for kt in range(5):
                  t = wfpool2.tile([KT_ROWS[kt], NKPL * 128], BF16,
                                   name="t", tag=f"wf{kt}")
                  nc.sync.dma_start(
                      t[:], wfT[r0:r0 + KT_ROWS[kt], c0:c0 + NKPL * 128])
                  wf_t.append(t)
                  r0 += KT_ROWS[kt]

              pst = [ps_tv.tile([128, NB2], F32, name="t", tag=f"tvps{ch}")
                     for ch in range(NCHUNK)]
              prods = []
              id_pending = None   # (prod, start) one tap behind, so the PE
              # never FIFO-stalls on the DVE mult it depends on
              for kpl in range(NKPL):
                  di, dj = kpl // 3, kpl % 3
                  psf = ps_f.tile([128, NIJ], F32, name="t", tag="fc")
                  for kt in range(5):
                      nc.tensor.matmul(
                          psf[:],
                          wf_t[kt][:, 128 * kpl:128 * (kpl + 1)],
                          p3[kt][:],
                          start=(kt == 0), stop=(kt == 4))
                  wgt_t = wgtpool.tile([128, NIJ], BF16, name="t", tag="wgt")
                  nc.scalar.activation(wgt_t[:], psf[:], AF.Copy)

                  if id_pending is not None:
                      p, st = id_pending
                      for ch in range(NCHUNK):
                          nc.tensor.matmul(
                              pst[ch][:], ident[:],
                              p[:, NB2 * ch:NB2 * (ch + 1)],
                              start=st, stop=False)
                      id_pending = None

                  # tvconv partial product for this tap, all 8 batches
                  wgb = (wgt_t[:].rearrange("p (i j) -> p i j", i=HP, j=HP)
                         .unsqueeze(1).broadcast_to((128, BPC, HP, HP)))
                  hwin = h_sb[ct][:, :, di:di + HP, dj:dj + HP]
                  prod = prodpool.tile([128, BPC * NIJ], BF16,
                                       name="t", tag=f"prod{kpl}")
                  pr = prod[:].rearrange(
                      "p (b i j) -> p b i j", b=BPC, i=HP, j=HP)
                  eng = nc.gpsimd if kpl in POOL_TAPS else nc.vector
                  eng.tensor_mul(pr, hwin, wgb)
                  if kpl < NKPL - 4 or ct in (5, 11):
                      id_pending = (prod, kpl == 0)
                  else:
                      prods.append(prod)
              if ct in (5, 11):
                  # all taps went through the PE: flush the pending tap with
                  # the group-closing stop flag
                  p, st = id_pending
                  for ch in range(NCHUNK):
                      nc.tensor.matmul(
                          pst[ch][:], ident[:],
                          p[:, NB2 * ch:NB2 * (ch + 1)],
                          start=st, stop=True)
                      nc.scalar.activation(
                          tvacc[ct][:, NB2 * ch:NB2 * (ch + 1)], pst[ch][:],
                          AF.Copy)
              else:
                  if id_pending is not None:
                      p, st = id_pending
                      for ch in range(NCHUNK):
                          nc.tensor.matmul(
                              pst[ch][:], ident[:],
                              p[:, NB2 * ch:NB2 * (ch + 1)],
                              start=st, stop=False)
                  # taps 5+6 and 7+8 pair-sum on DVE (engine balance: PE is
                  # the bottleneck), then two final identity-matmul
                  # accumulations
                  nc.vector.tensor_add(prods[0][:], prods[0][:], prods[1][:])
                  nc.vector.tensor_add(prods[2][:], prods[2][:], prods[3][:])
                  for ch in range(NCHUNK):
                      nc.tensor.matmul(
                          pst[ch][:], ident[:],
                          prods[0][:, NB2 * ch:NB2 * (ch + 1)],
                          start=False, stop=False)
                      nc.tensor.matmul(
                          pst[ch][:], ident[:],
                          prods[2][:, NB2 * ch:NB2 * (ch + 1)],
                          start=False, stop=True)
                      nc.scalar.activation(
                          tvacc[ct][:, NB2 * ch:NB2 * (ch + 1)], pst[ch][:],
                          AF.Copy)

              # gate as soon as the x2 half of a pair is done. gelu on ACT;
              # the gate multiply rides on Pool via scalar_tensor_tensor
              # (2D operands, cheaper there than tensor_tensor) to relieve
              # the DVE, which carries most of the tap products
              if ct >= NGT:
                  i = ct - NGT
                  ga = prodpool.tile([128, BPC * NIJ], BF16, name="t",
                                     tag="ga", bufs=3)
                  nc.scalar.activation(ga[:], tvacc[i][:], AF.Gelu)
                  nc.gpsimd.scalar_tensor_tensor(
                      tvacc[ct][:], ga[:], 1.0, tvacc[ct][:],
                      op0=OP.mult, op1=OP.mult)

          # ---------------- proj_out: W_out @ gated ----------------
          for m in range(2):
              for ch in range(NCHUNK):
                  ps = ps_proj.tile([128, NB2], F32, name="t", tag="pj")
                  for kt in range(NGT):
                      nc.tensor.matmul(
                          ps[:],
                          wout_sb[kt][:, 128 * m:128 * (m + 1)],
                          tvacc[NGT + kt][:, NB2 * ch:NB2 * (ch + 1)],
                          start=(kt == 0), stop=(kt == NGT - 1))
                  ot = outpool.tile([128, NB2], F32, name="t", tag="ot")
                  nc.scalar.activation(ot[:], ps[:], AF.Copy)
                  nc.sync.dma_start(
                      out_f[128 * m:128 * (m + 1), NB2 * ch:NB2 * (ch + 1)],
                      ot[:])

        for _rep in range(reps):
            emit_body()

    nc.compile()
    return nc


def _pack_shared(inputs):
    """Pack the batch-independent tensors (host-side layout marshalling)."""
    W_in = np.asarray(inputs["W_in"], np.float32)
    W_out = np.asarray(inputs["W_out"], np.float32)
    posi = np.asarray(inputs["posi_map"], np.float32)
    w0 = np.asarray(inputs["w0"], np.float32)
    w1 = np.asarray(inputs["w1"], np.float32)
    w2 = np.asarray(inputs["w2"], np.float32)
    wf = np.asarray(inputs["wf"], np.float32)

    padc = np.arange(CH)
    padc = np.where(padc < HID, padc, padc + (HIDP - HID))

    winP = np.zeros((CHP, DIM), np.float32)
    winP[padc] = W_in
    winT = np.ascontiguousarray(winP.T).astype(ml_dtypes.bfloat16)

    w0T = np.ascontiguousarray(
        w0.transpose(1, 2, 3, 0).reshape(4, 9, INTER)).astype(ml_dtypes.bfloat16)
    w1T = np.ascontiguousarray(
        w1.transpose(1, 2, 3, 0).reshape(INTER, 9, INTER)
    ).astype(ml_dtypes.bfloat16)
    w2T = np.ascontiguousarray(
        w2.transpose(1, 2, 3, 0).reshape(INTER, 9, INTER)
    ).astype(ml_dtypes.bfloat16)

    posiP = np.zeros((4, PH, PH), np.float32)
    posiP[:, 1:15, 1:15] = posi[0]
    posiP = posiP.astype(ml_dtypes.bfloat16)

    gbs = [np.asarray(inputs[k], np.float32).reshape(INTER, NIJ)
           for k in ("g0", "b0", "g1", "b1", "g2", "b2")]
    gb = np.stack(gbs, axis=1)   # (64, 6, 196)

    # wfT[(kh,kw,cin) row, kpl*CHP + padc] = wf[c*9+kpl, cin, kh, kw]
    wf5 = wf.reshape(CH, NKPL, INTER, 3, 3)
    wf5 = wf5.transpose(3, 4, 2, 1, 0)          # (kh, kw, cin, kpl, c)
    wfTp = np.zeros((576, NKPL, CHP), np.float32)
    wfTp[:, :, padc] = wf5.reshape(576, NKPL, CH)
    # ct-major column order: [ct, kpl, 128]
    wfTp = wfTp.reshape(576, NKPL, NCT, 128).transpose(0, 2, 1, 3)
    wfT = np.ascontiguousarray(
        wfTp.reshape(576, NKPL * CHP)).astype(ml_dtypes.bfloat16)

    woP = np.zeros((HIDP, DIM), np.float32)
    woP[:HID] = W_out.T
    woutT = woP.astype(ml_dtypes.bfloat16)

    return dict(winT=winT, posiP=posiP, w0T=w0T, w1T=w1T, w2T=w2T,
                gb=np.ascontiguousarray(gb), wfT=wfT, woutT=woutT,
                identD=np.eye(128, dtype=ml_dtypes.bfloat16))


def kernel(**inputs) -> np.ndarray:
    if "nc" not in _CACHE:
        _CACHE["nc"] = _build_nc()
    nc = _CACHE["nc"]

    x = np.asarray(inputs["x"], np.float32)     # (64, 256, 14, 14)
    shared = _pack_shared(inputs)

    in_maps = []
    for c in range(NCORES):
        xc = x[BPC * c:BPC * (c + 1)]           # (8, 256, 14, 14)
        xT = np.ascontiguousarray(
            xc.transpose(1, 0, 2, 3).reshape(DIM, BPC * NIJ)
        ).astype(ml_dtypes.bfloat16)
        m = dict(shared)
        m["xT"] = xT
        in_maps.append(m)

    res = run_bass_kernel_spmd(nc, in_maps, list(range(NCORES)))
    outs = []
    for c in range(NCORES):
        o = res.results[c]["out_f"].reshape(DIM, BPC, HP, HP)
        outs.append(o.transpose(1, 0, 2, 3))
    return np.ascontiguousarray(np.concatenate(outs, axis=0), dtype=np.float32)


# revision 39
# speedup vs baseline: 1.1551x; 1.0630x over previous
"""FRFN forward kernel for 8 Trainium2 NeuronCores.

Sharding: pure data parallel over batch B=64 -> 8 batches per core.
The TVConv generated weight (batch-independent) is recomputed on every
core (its cost hides under the DVE-bound tvconv stage).

Per-core pipeline (all channel dims padded so x1/x2 halves align):
  proj_in  : h = W_in @ x          PE bf16, output bf16 in padded
                                   (128, 8, 16, 16) spatial layout
  weightgen: 3x (3x3 conv + LayerNorm(CHW) + relu) on 4x14x14 posi map
             (fp32, tiny) then final conv -> wgt (CH*9, 196)  PE bf16
  tvconv   : out[c,b,ij] = sum_k wgt[c,k,ij] * h[c,b,ij@k]    DVE+Pool
  gate     : gelu(x1) * x2                                    ACT+Pool
  proj_out : W_out @ gated                                    PE bf16
"""

import numpy as np
import ml_dtypes
from contextlib import ExitStack

import concourse.bacc as bacc
import concourse.bass as bass
import concourse.mybir as mybir
import concourse.tile as tile
from concourse.bass_utils import run_bass_kernel_spmd

F32 = mybir.dt.float32
BF16 = mybir.dt.bfloat16
AF = mybir.ActivationFunctionType
OP = mybir.AluOpType

NCORES = 8
B = 64
BPC = B // NCORES          # 8 batches per core
DIM = 256
HID = 680
CH = 2 * HID               # 1360
HIDP = 768                 # padded x1/x2 half (6 * 128)
CHP = 2 * HIDP             # 1536
NCT = CHP // 128           # 12 channel tiles
NGT = HIDP // 128          # 6 gate tiles
HP = 14
NIJ = HP * HP              # 196
PH = 16                    # padded spatial side
PHW = PH * PH              # 256
INTER = 64
NKPL = 9                   # 3x3 taps
KT_ROWS = [128, 128, 128, 128, 64]   # 576 contraction rows for 3x3xINTER convs
NCHUNK = 4                 # N chunks for the big matmuls (2 batches x 196)
NB2 = 2 * NIJ              # 392
EPS = 1e-5
NLN = float(INTER * NIJ)   # layernorm normalizes over (C,H,W) = 64*196

# tap mults offloaded from DVE to the otherwise-idle Pool engine; their
# wgt is ready early in the per-tile stream and Pool has ~6.4us of
# serial time for the two of them inside an ~11us tile stage
POOL_TAPS = ()

_CACHE = {}


def _build_nc(reps=1):
    nc = bacc.Bacc("TRN2", target_bir_lowering=False)

    xT = nc.dram_tensor("xT", [DIM, BPC * NIJ], BF16, kind="ExternalInput")
    winT = nc.dram_tensor("winT", [DIM, CHP], BF16, kind="ExternalInput")
    posiP = nc.dram_tensor("posiP", [4, PH, PH], BF16, kind="ExternalInput")
    w0T = nc.dram_tensor("w0T", [4, NKPL, INTER], BF16, kind="ExternalInput")
    w1T = nc.dram_tensor("w1T", [INTER, NKPL, INTER], BF16,
                         kind="ExternalInput")
    w2T = nc.dram_tensor("w2T", [INTER, NKPL, INTER], BF16,
                         kind="ExternalInput")
    gb = nc.dram_tensor("gb", [INTER, 6, NIJ], F32, kind="ExternalInput")
    wfT = nc.dram_tensor("wfT", [576, NKPL * CHP], BF16, kind="ExternalInput")
    woutT = nc.dram_tensor("woutT", [HIDP, DIM], BF16, kind="ExternalInput")
    identD = nc.dram_tensor("identD", [128, 128], BF16, kind="ExternalInput")
    out_f = nc.dram_tensor("out_f", [DIM, BPC * NIJ], F32, kind="ExternalOutput")

    with tile.TileContext(nc) as tc, ExitStack() as ctx:
        persist = ctx.enter_context(tc.tile_pool(name="persist", bufs=1))
        work = ctx.enter_context(tc.tile_pool(name="work", bufs=2))
        wgtpool = ctx.enter_context(tc.tile_pool(name="wgtpool", bufs=9))
        prodpool = ctx.enter_context(tc.tile_pool(name="prodpool", bufs=1))
        outpool = ctx.enter_context(tc.tile_pool(name="outpool", bufs=4))
        wfpool2 = ctx.enter_context(tc.tile_pool(name="wfpool2", bufs=2))
        ps_proj = ctx.enter_context(
            tc.tile_pool(name="ps_proj", bufs=2, space="PSUM"))
        ps_f = ctx.enter_context(
            tc.tile_pool(name="ps_f", bufs=2, space="PSUM"))
        ps_tv = ctx.enter_context(
            tc.tile_pool(name="ps_tv", bufs=1, space="PSUM"))
        ps_s = ps_f

        # ---------------- persistent SBUF tensors ----------------
        h_sb = [persist.tile([128, BPC, PH, PH], BF16, name="t", tag=f"h{i}")
                for i in range(NCT)]
        tvacc = [persist.tile([128, BPC * NIJ], BF16, name="t", tag=f"tv{i}")
                 for i in range(NCT)]
        wout_sb = [persist.tile([128, DIM], BF16, name="t", tag=f"wo{i}")
                   for i in range(NGT)]

        # small-conv chain buffers
        posi_sb = persist.tile([4, PH, PH], BF16, name="t", tag="posi")
        w0_sb = persist.tile([4, NKPL, INTER], BF16, name="t", tag="w0")
        w1_sb = persist.tile([INTER, NKPL, INTER], BF16, name="t", tag="w1")
        w2_sb = persist.tile([INTER, NKPL, INTER], BF16, name="t", tag="w2")
        gb_sb = persist.tile([INTER, 6, NIJ], F32, name="t", tag="gb")
        pad1 = persist.tile([INTER, PH, PH], BF16, name="t", tag="pad1")
        pad2 = persist.tile([INTER, PH, PH], BF16, name="t", tag="pad2")
        pad3 = persist.tile([INTER, PH, PH], BF16, name="t", tag="pad3")
        p3 = [persist.tile([KT_ROWS[k], NIJ], BF16, name="t", tag=f"p3_{k}")
              for k in range(5)]
        ones_c = persist.tile([INTER, 1], F32, name="t", tag="ones_c")
        ones_r = persist.tile([1, INTER], F32, name="t", tag="ones_r")
        ident = persist.tile([128, 128], BF16, name="t", tag="ident")
        eps_t = persist.tile([1, 1], F32, name="t", tag="eps")

        # ---------------- input DMAs + memsets ----------------
        nc.sync.dma_start(posi_sb[:], posiP[:])
        nc.sync.dma_start(ident[:], identD[:])
        nc.sync.dma_start(w0_sb[:], w0T[:])
        nc.sync.dma_start(w1_sb[:], w1T[:])
        nc.sync.dma_start(w2_sb[:], w2T[:])
        nc.sync.dma_start(gb_sb[:], gb[:])
        for i in range(NGT):
            nc.sync.dma_start(wout_sb[i][:], woutT[128 * i:128 * (i + 1), :])

        # pre-warm the ACT function tables off the critical path (table
        # switches cost ~1.3us each and would otherwise fire mid-LN-chain)
        warm = persist.tile([1, 1], F32, name="t", tag="warm")
        nc.gpsimd.memset(warm[:], 1.0)
        wsink = persist.tile([1, 1], F32, name="t", tag="wsink")
        for fn in (AF.Sqrt, AF.Relu, AF.Gelu, AF.Identity):
            nc.scalar.activation(wsink[:], warm[:], fn)

        nc.gpsimd.memset(ones_c[:], 1.0)
        nc.gpsimd.memset(eps_t[:], EPS)
        nc.gpsimd.memset(ones_r[:], 1.0)
        nc.gpsimd.memset(pad1[:], 0.0)
        nc.gpsimd.memset(pad2[:], 0.0)
        nc.gpsimd.memset(pad3[:], 0.0)
        for i in range(NCT):
            # zero only the pad borders (the 14x14 interior gets overwritten
            # by the proj_in drains)
            t = h_sb[i]
            nc.gpsimd.memset(t[:, :, 0, :], 0.0)
            nc.gpsimd.memset(t[:, :, 15, :], 0.0)
            nc.gpsimd.memset(t[:, :, 1:15, 0], 0.0)
            nc.gpsimd.memset(t[:, :, 1:15, 15], 0.0)

        def emit_body():
          # ------------- weight-gen small conv chain (fp32) -------------
          def layernorm_relu(ps_in, g_ap, b_ap, pad_tile):
              """ps_in: PSUM (64,196) conv output. Writes relu(LN(x)*g+b) into
              pad_tile[:, 1:15, 1:15] (borders stay zero)."""
              sq = work.tile([INTER, NIJ], F32, name="t", tag="ln_sq")
              hval = work.tile([INTER, NIJ], F32, name="t", tag="ln_h")
              stats = work.tile([INTER, 2], F32, name="t", tag="ln_st")
              nc.scalar.activation(sq[:], ps_in[:], AF.Square,
                                   accum_out=stats[:, 1:2])
              nc.scalar.activation(hval[:], ps_in[:], AF.Copy,
                                   accum_out=stats[:, 0:1])
              # cross-partition reduce: [sum; sumsq] = ones.T @ stats
              ps_r = ps_s.tile([1, 2], F32, name="t", tag="fc")
              nc.tensor.matmul(ps_r[:], ones_c[:], stats[:],
                               start=True, stop=True)
              mr = work.tile([1, 2], F32, name="t", tag="ln_mr")
              musq = work.tile([1, 1], F32, name="t", tag="ln_musq")
              nc.scalar.activation(musq[:], ps_r[:, 0:1], AF.Square,
                                   scale=1.0 / NLN)
              e2e = work.tile([1, 1], F32, name="t", tag="ln_e2e")
              nc.scalar.activation(e2e[:], ps_r[:, 1:2], AF.Identity,
                                   scale=1.0 / NLN, bias=eps_t[:])
              nc.scalar.activation(mr[:, 0:1], ps_r[:, 0:1], AF.Copy,
                                   scale=1.0 / NLN)
              std = work.tile([1, 1], F32, name="t", tag="ln_std")
              nc.scalar.activation(std[:], musq[:], AF.Sqrt,
                                   scale=-1.0, bias=e2e[:])
              nc.vector.reciprocal(mr[:, 1:2], std[:])
              # broadcast [mu, rstd] to all 64 partitions via rank-1 matmul
              ps_bc = ps_s.tile([INTER, 2], F32, name="t", tag="fc")
              nc.tensor.matmul(ps_bc[:], ones_r[:], mr[:], start=True, stop=True)
              bc = work.tile([INTER, 2], F32, name="t", tag="ln_bc")
              nc.scalar.activation(bc[:], ps_bc[:], AF.Copy)
              xn = work.tile([INTER, NIJ], F32, name="t", tag="ln_xn")
              nc.vector.tensor_scalar(xn[:], hval[:], bc[:, 0:1], bc[:, 1:2],
                                      op0=OP.subtract, op1=OP.mult)
              t2 = work.tile([INTER, NIJ], F32, name="t", tag="ln_t2")
              nc.vector.tensor_mul(t2[:], xn[:], g_ap)
              t3 = work.tile([INTER, NIJ], F32, name="t", tag="ln_t3")
              nc.vector.tensor_add(t3[:], t2[:], b_ap)
              dst = pad_tile[:, 1:15, 1:15]
              src = t3[:].rearrange("p (i j) -> p i j", i=HP, j=HP)
              nc.scalar.activation(dst, src, AF.Relu)

          def conv3x3(w_sb, pad_tile, ps_out):
              """3x3 conv as 9 accumulating matmuls over shifted windows."""
              for kap in range(NKPL):
                  di, dj = kap // 3, kap % 3
                  nc.tensor.matmul(ps_out[:], w_sb[:, kap, :],
                                   pad_tile[:, di:di + HP, dj:dj + HP],
                                   start=(kap == 0), stop=(kap == NKPL - 1))

          ps0 = ps_s.tile([INTER, NIJ], F32, name="t", tag="fc")
          conv3x3(w0_sb, posi_sb, ps0)
          layernorm_relu(ps0, gb_sb[:, 0, :], gb_sb[:, 1, :], pad1)
          ps1 = ps_s.tile([INTER, NIJ], F32, name="t", tag="fc")
          conv3x3(w1_sb, pad1, ps1)
          layernorm_relu(ps1, gb_sb[:, 2, :], gb_sb[:, 3, :], pad2)
          ps2 = ps_s.tile([INTER, NIJ], F32, name="t", tag="fc")
          conv3x3(w2_sb, pad2, ps2)
          layernorm_relu(ps2, gb_sb[:, 4, :], gb_sb[:, 5, :], pad3)
          # build the 576-row im2col for the final conv, 2 parallel queues
          qengs = [nc.sync, nc.scalar]
          for kt in range(5):
              nk = KT_ROWS[kt] // 64
              for sub in range(nk):
                  kap = 2 * kt + sub
                  di, dj = kap // 3, kap % 3
                  srcw = pad3[:, di:di + HP, dj:dj + HP]
                  dst = p3[kt][64 * sub:64 * (sub + 1), :]
                  dst = dst.rearrange("p (i j) -> p i j", i=HP, j=HP)
                  qengs[kap % 2].dma_start(dst, srcw)

          # ------- fused per-channel-tile loop: proj_in -> convf -> tvconv ----
          # wfT is packed ct-major: column ct*1152 + kpl*128 + p.
          # Per channel tile: proj_in matmuls fill the padded h tile; then 9
          # taps of conv-f -> wgt -> DVE/Pool product; the 9-tap sum runs on
          # the PE as identity-matmul PSUM accumulation (exact bf16 identity,
          # fp32 accumulate). Tiles visited in gate-pair order so gelu*gate
          # can fire as soon as a pair completes.
          x_sb = [persist.tile([128, BPC * NIJ], BF16, name="t", tag=f"x{i}")
                  for i in range(2)]
          win_sb = [persist.tile([128, CHP], BF16, name="t", tag=f"wi{i}")
                    for i in range(2)]
          for i in range(2):
              nc.sync.dma_start(x_sb[i][:], xT[128 * i:128 * (i + 1), :])
              nc.sync.dma_start(win_sb[i][:], winT[128 * i:128 * (i + 1), :])

          CT_ORDER = [0, 6, 1, 7, 2, 8, 3, 9, 4, 10, 5, 11]
          for ct in CT_ORDER:
              # proj_in for this channel tile
              for ch in range(NCHUNK):
                  ps = ps_proj.tile([128, NB2], F32, name="t", tag="pj")
                  for kt in range(2):
                      nc.tensor.matmul(
                          ps[:],
                          win_sb[kt][:, 128 * ct:128 * (ct + 1)],
                          x_sb[kt][:, NB2 * ch:NB2 * (ch + 1)],
                          start=(kt == 0), stop=(kt == 1))
                  # drain into padded (b, 16, 16) layout as bf16
                  dst = h_sb[ct][:, 2 * ch:2 * ch + 2, 1:15, 1:15]
                  src = ps[:].rearrange("p (b i j) -> p b i j",
                                        b=2, i=HP, j=HP)
                  nc.scalar.activation(dst, src, AF.Copy)

              # stream this tile's final-conv weights
              wf_t = []
              r0 = 0
              c0 = NKPL * 128 * ct
              for kt in range(5):
                  t = wfpool2.tile([KT_ROWS[kt], NKPL * 128], BF16,
                                   name="t", tag=f"wf{kt}")
                  nc.sync.dma_start(
                      t[:], wfT[r0:r0 + KT_ROWS[kt], c0:c0 + NKPL * 128])
                  wf_t.append(t)
                  r0 += KT_ROWS[kt]

              pst = [ps_tv.tile([128, NB2], F32, name="t", tag=f"tvps{ch}")
                     for ch in range(NCHUNK)]
              prods = []
              id_pending = None   # (prod, start) one tap behind, so the PE
              # never FIFO-stalls on the DVE mult it depends on
              for kpl in range(NKPL):
                  di, dj = kpl // 3, kpl % 3
                  psf = ps_f.tile([128, NIJ], F32, name="t", tag="fc")
                  for kt in range(5):
                      nc.tensor.matmul(
                          psf[:],
                          wf_t[kt][:, 128 * kpl:128 * (kpl + 1)],
                          p3[kt][:],
                          start=(kt == 0), stop=(kt == 4))
                  wgt_t = wgtpool.tile([128, NIJ], BF16, name="t", tag="wgt")
                  nc.scalar.activation(wgt_t[:], psf[:], AF.Copy)

                  if id_pending is not None:
                      p, st = id_pending
                      for ch in range(NCHUNK):
                          nc.tensor.matmul(
                              pst[ch][:], ident[:],
                              p[:, NB2 * ch:NB2 * (ch + 1)],
                              start=st, stop=False)
                      id_pending = None

                  # tvconv partial product for this tap, all 8 batches
                  wgb = (wgt_t[:].rearrange("p (i j) -> p i j", i=HP, j=HP)
                         .unsqueeze(1).broadcast_to((128, BPC, HP, HP)))
                  hwin = h_sb[ct][:, :, di:di + HP, dj:dj + HP]
                  prod = prodpool.tile([128, BPC * NIJ], BF16,
                                       name="t", tag=f"prod{kpl}")
                  pr = prod[:].rearrange(
                      "p (b i j) -> p b i j", b=BPC, i=HP, j=HP)
                  eng = nc.gpsimd if kpl in POOL_TAPS else nc.vector
                  eng.tensor_mul(pr, hwin, wgb)
                  if kpl < NKPL - 4 or ct in (5, 11):
                      id_pending = (prod, kpl == 0)
                  else:
                      prods.append(prod)
              if ct in (5, 11):
                  # all taps went through the PE: flush the pending tap with
                  # the group-closing stop flag
                  p, st = id_pending
                  for ch in range(NCHUNK):
                      nc.tensor.matmul(
                          pst[ch][:], ident[:],
                          p[:, NB2 * ch:NB2 * (ch + 1)],
                          start=st, stop=True)
                      nc.scalar.activation(
                          tvacc[ct][:, NB2 * ch:NB2 * (ch + 1)], pst[ch][:],
                          AF.Copy)
              else:
                  if id_pending is not None:
                      p, st = id_pending
                      for ch in range(NCHUNK):
                          nc.tensor.matmul(
                              pst[ch][:], ident[:],
                              p[:, NB2 * ch:NB2 * (ch + 1)],
                              start=st, stop=False)
                  # taps 5+6 and 7+8 pair-sum on DVE (engine balance: PE is
                  # the bottleneck), then two final identity-matmul
                  # accumulations
                  nc.vector.tensor_add(prods[0][:], prods[0][:], prods[1][:])
                  nc.vector.tensor_add(prods[2][:], prods[2][:], prods[3][:])
                  for ch in range(NCHUNK):
                      nc.tensor.matmul(
                          pst[ch][:], ident[:],
                          prods[0][:, NB2 * ch:NB2 * (ch + 1)],
                          start=False, stop=False)
                      nc.tensor.matmul(
                          pst[ch][:], ident[:],
                          prods[2][:, NB2 * ch:NB2 * (ch + 1)],
                          start=False, stop=True)
                      nc.scalar.activation(
                          tvacc[ct][:, NB2 * ch:NB2 * (ch + 1)], pst[ch][:],
                          AF.Copy)

              # gate as soon as the x2 half of a pair is done. gelu on ACT;
              # the gate multiply rides on Pool via scalar_tensor_tensor
              # (2D operands, cheaper there than tensor_tensor) to relieve
              # the DVE, which carries most of the tap products
              if ct >= NGT:
                  i = ct - NGT
                  ga = prodpool.tile([128, BPC * NIJ], BF16, name="t",
                                     tag="ga", bufs=3)
                  nc.scalar.activation(ga[:], tvacc[i][:], AF.Gelu)
                  nc.vector.tensor_mul(tvacc[ct][:], ga[:], tvacc[ct][:])

          # ---------------- proj_out: W_out @ gated ----------------
          for m in range(2):
              for ch in range(NCHUNK):
                  ps = ps_proj.tile([128, NB2], F32, name="t", tag="pj")
                  for kt in range(NGT):
                      nc.tensor.matmul(
                          ps[:],
                          wout_sb[kt][:, 128 * m:128 * (m + 1)],
                          tvacc[NGT + kt][:, NB2 * ch:NB2 * (ch + 1)],
                          start=(kt == 0), stop=(kt == NGT - 1))
                  ot = outpool.tile([128, NB2], F32, name="t", tag="ot")
                  nc.scalar.activation(ot[:], ps[:], AF.Copy)
                  nc.sync.dma_start(
                      out_f[128 * m:128 * (m + 1), NB2 * ch:NB2 * (ch + 1)],
                      ot[:])

        for _rep in range(reps):
            emit_body()

    nc.compile()
    return nc


def _pack_shared(inputs):
    """Pack the batch-independent tensors (host-side layout marshalling)."""
    W_in = np.asarray(inputs["W_in"], np.float32)
    W_out = np.asarray(inputs["W_out"], np.float32)
    posi = np.asarray(inputs["posi_map"], np.float32)
    w0 = np.asarray(inputs["w0"], np.float32)
    w1 = np.asarray(inputs["w1"], np.float32)
    w2 = np.asarray(inputs["w2"], np.float32)
    wf = np.asarray(inputs["wf"], np.float32)

    padc = np.arange(CH)
    padc = np.where(padc < HID, padc, padc + (HIDP - HID))

    winP = np.zeros((CHP, DIM), np.float32)
    winP[padc] = W_in
    winT = np.ascontiguousarray(winP.T).astype(ml_dtypes.bfloat16)

    w0T = np.ascontiguousarray(
        w0.transpose(1, 2, 3, 0).reshape(4, 9, INTER)).astype(ml_dtypes.bfloat16)
    w1T = np.ascontiguousarray(
        w1.transpose(1, 2, 3, 0).reshape(INTER, 9, INTER)
    ).astype(ml_dtypes.bfloat16)
    w2T = np.ascontiguousarray(
        w2.transpose(1, 2, 3, 0).reshape(INTER, 9, INTER)
    ).astype(ml_dtypes.bfloat16)

    posiP = np.zeros((4, PH, PH), np.float32)
    posiP[:, 1:15, 1:15] = posi[0]
    posiP = posiP.astype(ml_dtypes.bfloat16)

    gbs = [np.asarray(inputs[k], np.float32).reshape(INTER, NIJ)
           for k in ("g0", "b0", "g1", "b1", "g2", "b2")]
    gb = np.stack(gbs, axis=1)   # (64, 6, 196)

    # wfT[(kh,kw,cin) row, kpl*CHP + padc] = wf[c*9+kpl, cin, kh, kw]
    wf5 = wf.reshape(CH, NKPL, INTER, 3, 3)
    wf5 = wf5.transpose(3, 4, 2, 1, 0)          # (kh, kw, cin, kpl, c)
    wfTp = np.zeros((576, NKPL, CHP), np.float32)
    wfTp[:, :, padc] = wf5.reshape(576, NKPL, CH)
    # ct-major column order: [ct, kpl, 128]
    wfTp = wfTp.reshape(576, NKPL, NCT, 128).transpose(0, 2, 1, 3)
    wfT = np.ascontiguousarray(
        wfTp.reshape(576, NKPL * CHP)).astype(ml_dtypes.bfloat16)

    woP = np.zeros((HIDP, DIM), np.float32)
    woP[:HID] = W_out.T
    woutT = woP.astype(ml_dtypes.bfloat16)

    return dict(winT=winT, posiP=posiP, w0T=w0T, w1T=w1T, w2T=w2T,
                gb=np.ascontiguousarray(gb), wfT=wfT, woutT=woutT,
                identD=np.eye(128, dtype=ml_dtypes.bfloat16))


def kernel(**inputs) -> np.ndarray:
    if "nc" not in _CACHE:
        _CACHE["nc"] = _build_nc()
    nc = _CACHE["nc"]

    x = np.asarray(inputs["x"], np.float32)     # (64, 256, 14, 14)
    shared = _pack_shared(inputs)

    in_maps = []
    for c in range(NCORES):
        xc = x[BPC * c:BPC * (c + 1)]           # (8, 256, 14, 14)
        xT = np.ascontiguousarray(
            xc.transpose(1, 0, 2, 3).reshape(DIM, BPC * NIJ)
        ).astype(ml_dtypes.bfloat16)
        m = dict(shared)
        m["xT"] = xT
        in_maps.append(m)

    res = run_bass_kernel_spmd(nc, in_maps, list(range(NCORES)))
    outs = []
    for c in range(NCORES):
        o = res.results[c]["out_f"].reshape(DIM, BPC, HP, HP)
        outs.append(o.transpose(1, 0, 2, 3))
    return np.ascontiguousarray(np.concatenate(outs, axis=0), dtype=np.float32)


# revision 45
# speedup vs baseline: 1.1711x; 1.0138x over previous
"""FRFN forward kernel for 8 Trainium2 NeuronCores.

Sharding: pure data parallel over batch B=64 -> 8 batches per core.
The TVConv generated weight (batch-independent) is recomputed on every
core (its cost hides under the DVE-bound tvconv stage).

Per-core pipeline (all channel dims padded so x1/x2 halves align):
  proj_in  : h = W_in @ x          PE bf16, output bf16 in padded
                                   (128, 8, 16, 16) spatial layout
  weightgen: 3x (3x3 conv + LayerNorm(CHW) + relu) on 4x14x14 posi map
             (fp32, tiny) then final conv -> wgt (CH*9, 196)  PE bf16
  tvconv   : out[c,b,ij] = sum_k wgt[c,k,ij] * h[c,b,ij@k]    DVE+Pool
  gate     : gelu(x1) * x2                                    ACT+Pool
  proj_out : W_out @ gated                                    PE bf16
"""

import numpy as np
import ml_dtypes
from contextlib import ExitStack

import concourse.bacc as bacc
import concourse.bass as bass
import concourse.mybir as mybir
import concourse.tile as tile
from concourse.bass_utils import run_bass_kernel_spmd

F32 = mybir.dt.float32
BF16 = mybir.dt.bfloat16
AF = mybir.ActivationFunctionType
OP = mybir.AluOpType

NCORES = 8
B = 64
BPC = B // NCORES          # 8 batches per core
DIM = 256
HID = 680
CH = 2 * HID               # 1360
HIDP = 768                 # padded x1/x2 half (6 * 128)
CHP = 2 * HIDP             # 1536
NCT = CHP // 128           # 12 channel tiles
NGT = HIDP // 128          # 6 gate tiles
HP = 14
NIJ = HP * HP              # 196
PH = 16                    # padded spatial side
PHW = PH * PH              # 256
INTER = 64
NKPL = 9                   # 3x3 taps
KT_ROWS = [128, 128, 128, 128, 64]   # 576 contraction rows for 3x3xINTER convs
NCHUNK = 4                 # N chunks for the big matmuls (2 batches x 196)
NB2 = 2 * NIJ              # 392
EPS = 1e-5
NLN = float(INTER * NIJ)   # layernorm normalizes over (C,H,W) = 64*196

# tap mults offloaded from DVE to the otherwise-idle Pool engine; their
# wgt is ready early in the per-tile stream and Pool has ~6.4us of
# serial time for the two of them inside an ~11us tile stage
POOL_TAPS = ()

_CACHE = {}


def _build_nc(reps=1):
    nc = bacc.Bacc("TRN2", target_bir_lowering=False)

    xT = nc.dram_tensor("xT", [DIM, BPC * NIJ], BF16, kind="ExternalInput")
    winT = nc.dram_tensor("winT", [DIM, CHP], BF16, kind="ExternalInput")
    posiP = nc.dram_tensor("posiP", [4, PH, PH], BF16, kind="ExternalInput")
    w0T = nc.dram_tensor("w0T", [4, NKPL, INTER], BF16, kind="ExternalInput")
    w1T = nc.dram_tensor("w1T", [INTER, NKPL, INTER], BF16,
                         kind="ExternalInput")
    w2T = nc.dram_tensor("w2T", [INTER, NKPL, INTER], BF16,
                         kind="ExternalInput")
    gb = nc.dram_tensor("gb", [INTER, 6, NIJ], F32, kind="ExternalInput")
    wfT = nc.dram_tensor("wfT", [576, NKPL * CHP], BF16, kind="ExternalInput")
    woutT = nc.dram_tensor("woutT", [HIDP, DIM], BF16, kind="ExternalInput")
    identD = nc.dram_tensor("identD", [128, 128], BF16, kind="ExternalInput")
    out_f = nc.dram_tensor("out_f", [DIM, BPC * NIJ], F32, kind="ExternalOutput")

    with tile.TileContext(nc) as tc, ExitStack() as ctx:
        persist = ctx.enter_context(tc.tile_pool(name="persist", bufs=1))
        work = ctx.enter_context(tc.tile_pool(name="work", bufs=2))
        wgtpool = ctx.enter_context(tc.tile_pool(name="wgtpool", bufs=9))
        prodpool = ctx.enter_context(tc.tile_pool(name="prodpool", bufs=1))
        outpool = ctx.enter_context(tc.tile_pool(name="outpool", bufs=4))
        wfpool2 = ctx.enter_context(tc.tile_pool(name="wfpool2", bufs=2))
        ps_proj = ctx.enter_context(
            tc.tile_pool(name="ps_proj", bufs=2, space="PSUM"))
        ps_f = ctx.enter_context(
            tc.tile_pool(name="ps_f", bufs=2, space="PSUM"))
        ps_tv = ctx.enter_context(
            tc.tile_pool(name="ps_tv", bufs=1, space="PSUM"))
        ps_s = ps_f

        # ---------------- persistent SBUF tensors ----------------
        h_sb = [persist.tile([128, BPC, PH, PH], BF16, name="t", tag=f"h{i}")
                for i in range(NCT)]
        tvacc = [persist.tile([128, BPC * NIJ], BF16, name="t", tag=f"tv{i}")
                 for i in range(NCT)]
        wout_sb = [persist.tile([128, DIM], BF16, name="t", tag=f"wo{i}")
                   for i in range(NGT)]

        # small-conv chain buffers
        posi_sb = persist.tile([4, PH, PH], BF16, name="t", tag="posi")
        w0_sb = persist.tile([4, NKPL, INTER], BF16, name="t", tag="w0")
        w1_sb = persist.tile([INTER, NKPL, INTER], BF16, name="t", tag="w1")
        w2_sb = persist.tile([INTER, NKPL, INTER], BF16, name="t", tag="w2")
        gb_sb = persist.tile([INTER, 6, NIJ], F32, name="t", tag="gb")
        pad1 = persist.tile([INTER, PH, PH], BF16, name="t", tag="pad1")
        pad2 = persist.tile([INTER, PH, PH], BF16, name="t", tag="pad2")
        pad3 = persist.tile([INTER, PH, PH], BF16, name="t", tag="pad3")
        p3 = [persist.tile([KT_ROWS[k], NIJ], BF16, name="t", tag=f"p3_{k}")
              for k in range(5)]
        ones_c = persist.tile([INTER, 1], F32, name="t", tag="ones_c")
        ones_r = persist.tile([1, INTER], F32, name="t", tag="ones_r")
        ident = persist.tile([128, 128], BF16, name="t", tag="ident")
        eps_t = persist.tile([1, 1], F32, name="t", tag="eps")

        # ---------------- input DMAs + memsets ----------------
        nc.sync.dma_start(posi_sb[:], posiP[:])
        nc.sync.dma_start(ident[:], identD[:])
        nc.sync.dma_start(w0_sb[:], w0T[:])
        nc.sync.dma_start(w1_sb[:], w1T[:])
        nc.sync.dma_start(w2_sb[:], w2T[:])
        nc.sync.dma_start(gb_sb[:], gb[:])
        for i in range(NGT):
            nc.sync.dma_start(wout_sb[i][:], woutT[128 * i:128 * (i + 1), :])

        # pre-warm the ACT function tables off the critical path (table
        # switches cost ~1.3us each and would otherwise fire mid-LN-chain)
        warm = persist.tile([1, 1], F32, name="t", tag="warm")
        nc.gpsimd.memset(warm[:], 1.0)
        wsink = persist.tile([1, 1], F32, name="t", tag="wsink")
        for fn in (AF.Sqrt, AF.Relu, AF.Gelu, AF.Identity):
            nc.scalar.activation(wsink[:], warm[:], fn)

        nc.gpsimd.memset(ones_c[:], 1.0)
        nc.gpsimd.memset(eps_t[:], EPS)
        nc.gpsimd.memset(ones_r[:], 1.0)
        nc.gpsimd.memset(pad1[:], 0.0)
        nc.gpsimd.memset(pad2[:], 0.0)
        nc.gpsimd.memset(pad3[:], 0.0)
        for i in range(NCT):
            # zero only the pad borders (the 14x14 interior gets overwritten
            # by the proj_in drains)
            t = h_sb[i]
            nc.gpsimd.memset(t[:, :, 0, :], 0.0)
            nc.gpsimd.memset(t[:, :, 15, :], 0.0)
            nc.gpsimd.memset(t[:, :, 1:15, 0], 0.0)
            nc.gpsimd.memset(t[:, :, 1:15, 15], 0.0)

        def emit_body():
          # ------------- weight-gen small conv chain (fp32) -------------
          def layernorm_relu(ps_in, g_ap, b_ap, pad_tile):
              """ps_in: PSUM (64,196) conv output. Writes relu(LN(x)*g+b) into
              pad_tile[:, 1:15, 1:15] (borders stay zero)."""
              sq = work.tile([INTER, NIJ], F32, name="t", tag="ln_sq")
              hval = work.tile([INTER, NIJ], F32, name="t", tag="ln_h")
              stats = work.tile([INTER, 2], F32, name="t", tag="ln_st")
              nc.scalar.activation(sq[:], ps_in[:], AF.Square,
                                   accum_out=stats[:, 1:2])
              nc.scalar.activation(hval[:], ps_in[:], AF.Copy,
                                   accum_out=stats[:, 0:1])
              # cross-partition reduce: [sum; sumsq] = ones.T @ stats
              ps_r = ps_s.tile([1, 2], F32, name="t", tag="fc")
              nc.tensor.matmul(ps_r[:], ones_c[:], stats[:],
                               start=True, stop=True)
              mr = work.tile([1, 2], F32, name="t", tag="ln_mr")
              musq = work.tile([1, 1], F32, name="t", tag="ln_musq")
              nc.scalar.activation(musq[:], ps_r[:, 0:1], AF.Square,
                                   scale=1.0 / NLN)
              e2e = work.tile([1, 1], F32, name="t", tag="ln_e2e")
              nc.scalar.activation(e2e[:], ps_r[:, 1:2], AF.Identity,
                                   scale=1.0 / NLN, bias=eps_t[:])
              nc.scalar.activation(mr[:, 0:1], ps_r[:, 0:1], AF.Copy,
                                   scale=1.0 / NLN)
              std = work.tile([1, 1], F32, name="t", tag="ln_std")
              nc.scalar.activation(std[:], musq[:], AF.Sqrt,
                                   scale=-1.0, bias=e2e[:])
              nc.vector.reciprocal(mr[:, 1:2], std[:])
              # broadcast [mu, rstd] to all 64 partitions via rank-1 matmul
              ps_bc = ps_s.tile([INTER, 2], F32, name="t", tag="fc")
              nc.tensor.matmul(ps_bc[:], ones_r[:], mr[:], start=True, stop=True)
              bc = work.tile([INTER, 2], F32, name="t", tag="ln_bc")
              nc.scalar.activation(bc[:], ps_bc[:], AF.Copy)
              xn = work.tile([INTER, NIJ], F32, name="t", tag="ln_xn")
              nc.vector.tensor_scalar(xn[:], hval[:], bc[:, 0:1], bc[:, 1:2],
                                      op0=OP.subtract, op1=OP.mult)
              t2 = work.tile([INTER, NIJ], F32, name="t", tag="ln_t2")
              nc.vector.tensor_mul(t2[:], xn[:], g_ap)
              t3 = work.tile([INTER, NIJ], F32, name="t", tag="ln_t3")
              nc.vector.tensor_add(t3[:], t2[:], b_ap)
              dst = pad_tile[:, 1:15, 1:15]
              src = t3[:].rearrange("p (i j) -> p i j", i=HP, j=HP)
              nc.scalar.activation(dst, src, AF.Relu)

          def conv3x3(w_sb, pad_tile, ps_out):
              """3x3 conv as 9 accumulating matmuls over shifted windows."""
              for kap in range(NKPL):
                  di, dj = kap // 3, kap % 3
                  nc.tensor.matmul(ps_out[:], w_sb[:, kap, :],
                                   pad_tile[:, di:di + HP, dj:dj + HP],
                                   start=(kap == 0), stop=(kap == NKPL - 1))

          ps0 = ps_s.tile([INTER, NIJ], F32, name="t", tag="fc")
          conv3x3(w0_sb, posi_sb, ps0)
          layernorm_relu(ps0, gb_sb[:, 0, :], gb_sb[:, 1, :], pad1)
          ps1 = ps_s.tile([INTER, NIJ], F32, name="t", tag="fc")
          conv3x3(w1_sb, pad1, ps1)
          layernorm_relu(ps1, gb_sb[:, 2, :], gb_sb[:, 3, :], pad2)
          ps2 = ps_s.tile([INTER, NIJ], F32, name="t", tag="fc")
          conv3x3(w2_sb, pad2, ps2)
          layernorm_relu(ps2, gb_sb[:, 4, :], gb_sb[:, 5, :], pad3)
          # build the 576-row im2col for the final conv, 2 parallel queues
          qengs = [nc.sync, nc.scalar]
          for kt in range(5):
              nk = KT_ROWS[kt] // 64
              for sub in range(nk):
                  kap = 2 * kt + sub
                  di, dj = kap // 3, kap % 3
                  srcw = pad3[:, di:di + HP, dj:dj + HP]
                  dst = p3[kt][64 * sub:64 * (sub + 1), :]
                  dst = dst.rearrange("p (i j) -> p i j", i=HP, j=HP)
                  qengs[kap % 2].dma_start(dst, srcw)

          # ------- fused per-channel-tile loop: proj_in -> convf -> tvconv ----
          # wfT is packed ct-major: column ct*1152 + kpl*128 + p.
          # Per channel tile: proj_in matmuls fill the padded h tile; then 9
          # taps of conv-f -> wgt -> DVE/Pool product; the 9-tap sum runs on
          # the PE as identity-matmul PSUM accumulation (exact bf16 identity,
          # fp32 accumulate). Tiles visited in gate-pair order so gelu*gate
          # can fire as soon as a pair completes.
          x_sb = [persist.tile([128, BPC * NIJ], BF16, name="t", tag=f"x{i}")
                  for i in range(2)]
          win_sb = [persist.tile([128, CHP], BF16, name="t", tag=f"wi{i}")
                    for i in range(2)]
          for i in range(2):
              nc.sync.dma_start(x_sb[i][:], xT[128 * i:128 * (i + 1), :])
              nc.sync.dma_start(win_sb[i][:], winT[128 * i:128 * (i + 1), :])

          CT_ORDER = [0, 6, 1, 7, 2, 8, 3, 9, 4, 10, 5, 11]
          for ct in CT_ORDER:
              # proj_in for this channel tile
              for ch in range(NCHUNK):
                  ps = ps_proj.tile([128, NB2], F32, name="t", tag="pj")
                  for kt in range(2):
                      nc.tensor.matmul(
                          ps[:],
                          win_sb[kt][:, 128 * ct:128 * (ct + 1)],
                          x_sb[kt][:, NB2 * ch:NB2 * (ch + 1)],
                          start=(kt == 0), stop=(kt == 1))
                  # drain into padded (b, 16, 16) layout as bf16
                  dst = h_sb[ct][:, 2 * ch:2 * ch + 2, 1:15, 1:15]
                  src = ps[:].rearrange("p (b i j) -> p b i j",
                                        b=2, i=HP, j=HP)
                  nc.scalar.activation(dst, src, AF.Copy)

              # stream this tile's final-conv weights
              wf_t = []
              r0 = 0
              c0 = NKPL * 128 * ct
              for kt in range(5):
                  t = wfpool2.tile([KT_ROWS[kt], NKPL * 128], BF16,
                                   name="t", tag=f"wf{kt}")
                  nc.sync.dma_start(
                      t[:], wfT[r0:r0 + KT_ROWS[kt], c0:c0 + NKPL * 128])
                  wf_t.append(t)
                  r0 += KT_ROWS[kt]

              pst = [ps_tv.tile([128, NB2], F32, name="t", tag=f"tvps{ch}")
                     for ch in range(NCHUNK)]
              prods = []
              id_pending = None   # (prod, start) one tap behind, so the PE
              # never FIFO-stalls on the DVE mult it depends on
              for kpl in range(NKPL):
                  di, dj = kpl // 3, kpl % 3
                  psf = ps_f.tile([128, NIJ], F32, name="t", tag="fc")
                  for kt in range(5):
                      nc.tensor.matmul(
                          psf[:],
                          wf_t[kt][:, 128 * kpl:128 * (kpl + 1)],
                          p3[kt][:],
                          start=(kt == 0), stop=(kt == 4))
                  wgt_t = wgtpool.tile([128, NIJ], BF16, name="t", tag="wgt")
                  nc.scalar.activation(wgt_t[:], psf[:], AF.Copy)

                  if id_pending is not None:
                      p, st = id_pending
                      for ch in range(NCHUNK):
                          nc.tensor.matmul(
                              pst[ch][:], ident[:],
                              p[:, NB2 * ch:NB2 * (ch + 1)],
                              start=st, stop=False)
                      id_pending = None

                  # tvconv partial product for this tap: batches 0-6 on
                  # DVE, batch 7 on the otherwise-idle Pool engine. This
                  # shortens the DVE serial chain per tap (~0.78us vs
                  # 0.88us) that feeds the PE identity flushes.
                  wgb7 = (wgt_t[:].rearrange("p (i j) -> p i j", i=HP, j=HP)
                          .unsqueeze(1).broadcast_to((128, BPC - 1, HP, HP)))
                  wgb1 = (wgt_t[:].rearrange("p (i j) -> p i j", i=HP, j=HP)
                          .unsqueeze(1).broadcast_to((128, 1, HP, HP)))
                  hwin = h_sb[ct][:, :, di:di + HP, dj:dj + HP]
                  prod = prodpool.tile([128, BPC * NIJ], BF16,
                                       name="t", tag=f"prod{kpl}")
                  pr = prod[:].rearrange(
                      "p (b i j) -> p b i j", b=BPC, i=HP, j=HP)
                  nc.gpsimd.tensor_mul(pr[:, BPC - 1:BPC],
                                       hwin[:, BPC - 1:BPC], wgb1)
                  nc.vector.tensor_mul(pr[:, 0:BPC - 1],
                                       hwin[:, 0:BPC - 1], wgb7)
                  if kpl < NKPL - 4 or ct in (5, 11):
                      id_pending = (prod, kpl == 0)
                  else:
                      prods.append(prod)
              if ct in (5, 11):
                  # all taps went through the PE: flush the pending tap with
                  # the group-closing stop flag
                  p, st = id_pending
                  for ch in range(NCHUNK):
                      nc.tensor.matmul(
                          pst[ch][:], ident[:],
                          p[:, NB2 * ch:NB2 * (ch + 1)],
                          start=st, stop=True)
                      nc.scalar.activation(
                          tvacc[ct][:, NB2 * ch:NB2 * (ch + 1)], pst[ch][:],
                          AF.Copy)
              else:
                  if id_pending is not None:
                      p, st = id_pending
                      for ch in range(NCHUNK):
                          nc.tensor.matmul(
                              pst[ch][:], ident[:],
                              p[:, NB2 * ch:NB2 * (ch + 1)],
                              start=st, stop=False)
                  # taps 5+6 and 7+8 pair-sum on DVE (engine balance: PE is
                  # the bottleneck), then two final identity-matmul
                  # accumulations
                  nc.vector.tensor_add(prods[0][:], prods[0][:], prods[1][:])
                  nc.vector.tensor_add(prods[2][:], prods[2][:], prods[3][:])
                  for ch in range(NCHUNK):
                      nc.tensor.matmul(
                          pst[ch][:], ident[:],
                          prods[0][:, NB2 * ch:NB2 * (ch + 1)],
                          start=False, stop=False)
                      nc.tensor.matmul(
                          pst[ch][:], ident[:],
                          prods[2][:, NB2 * ch:NB2 * (ch + 1)],
                          start=False, stop=True)
                      nc.scalar.activation(
                          tvacc[ct][:, NB2 * ch:NB2 * (ch + 1)], pst[ch][:],
                          AF.Copy)

              # gate as soon as the x2 half of a pair is done. gelu on ACT;
              # the gate multiply rides on Pool via scalar_tensor_tensor
              # (2D operands, cheaper there than tensor_tensor) to relieve
              # the DVE, which carries most of the tap products
              if ct >= NGT:
                  i = ct - NGT
                  ga = prodpool.tile([128, BPC * NIJ], BF16, name="t",
                                     tag="ga", bufs=3)
                  nc.scalar.activation(ga[:], tvacc[i][:], AF.Gelu)
                  GS = (BPC - 1) * NIJ
                  nc.gpsimd.tensor_mul(tvacc[ct][:, GS:], ga[:, GS:],
                                       tvacc[ct][:, GS:])
                  nc.vector.tensor_mul(tvacc[ct][:, 0:GS], ga[:, 0:GS],
                                       tvacc[ct][:, 0:GS])

          # ---------------- proj_out: W_out @ gated ----------------
          for m in range(2):
              for ch in range(NCHUNK):
                  ps = ps_proj.tile([128, NB2], F32, name="t", tag="pj")
                  for kt in range(NGT):
                      nc.tensor.matmul(
                          ps[:],
                          wout_sb[kt][:, 128 * m:128 * (m + 1)],
                          tvacc[NGT + kt][:, NB2 * ch:NB2 * (ch + 1)],
                          start=(kt == 0), stop=(kt == NGT - 1))
                  ot = outpool.tile([128, NB2], F32, name="t", tag="ot")
                  nc.scalar.activation(ot[:], ps[:], AF.Copy)
                  nc.sync.dma_start(
                      out_f[128 * m:128 * (m + 1), NB2 * ch:NB2 * (ch + 1)],
                      ot[:])

        for _rep in range(reps):
            emit_body()

    nc.compile()
    return nc


def _pack_shared(inputs):
    """Pack the batch-independent tensors (host-side layout marshalling)."""
    W_in = np.asarray(inputs["W_in"], np.float32)
    W_out = np.asarray(inputs["W_out"], np.float32)
    posi = np.asarray(inputs["posi_map"], np.float32)
    w0 = np.asarray(inputs["w0"], np.float32)
    w1 = np.asarray(inputs["w1"], np.float32)
    w2 = np.asarray(inputs["w2"], np.float32)
    wf = np.asarray(inputs["wf"], np.float32)

    padc = np.arange(CH)
    padc = np.where(padc < HID, padc, padc + (HIDP - HID))

    winP = np.zeros((CHP, DIM), np.float32)
    winP[padc] = W_in
    winT = np.ascontiguousarray(winP.T).astype(ml_dtypes.bfloat16)

    w0T = np.ascontiguousarray(
        w0.transpose(1, 2, 3, 0).reshape(4, 9, INTER)).astype(ml_dtypes.bfloat16)
    w1T = np.ascontiguousarray(
        w1.transpose(1, 2, 3, 0).reshape(INTER, 9, INTER)
    ).astype(ml_dtypes.bfloat16)
    w2T = np.ascontiguousarray(
        w2.transpose(1, 2, 3, 0).reshape(INTER, 9, INTER)
    ).astype(ml_dtypes.bfloat16)

    posiP = np.zeros((4, PH, PH), np.float32)
    posiP[:, 1:15, 1:15] = posi[0]
    posiP = posiP.astype(ml_dtypes.bfloat16)

    gbs = [np.asarray(inputs[k], np.float32).reshape(INTER, NIJ)
           for k in ("g0", "b0", "g1", "b1", "g2", "b2")]
    gb = np.stack(gbs, axis=1)   # (64, 6, 196)

    # wfT[(kh,kw,cin) row, kpl*CHP + padc] = wf[c*9+kpl, cin, kh, kw]
    wf5 = wf.reshape(CH, NKPL, INTER, 3, 3)
    wf5 = wf5.transpose(3, 4, 2, 1, 0)          # (kh, kw, cin, kpl, c)
    wfTp = np.zeros((576, NKPL, CHP), np.float32)
    wfTp[:, :, padc] = wf5.reshape(576, NKPL, CH)
    # ct-major column order: [ct, kpl, 128]
    wfTp = wfTp.reshape(576, NKPL, NCT, 128).transpose(0, 2, 1, 3)
    wfT = np.ascontiguousarray(
        wfTp.reshape(576, NKPL * CHP)).astype(ml_dtypes.bfloat16)

    woP = np.zeros((HIDP, DIM), np.float32)
    woP[:HID] = W_out.T
    woutT = woP.astype(ml_dtypes.bfloat16)

    return dict(winT=winT, posiP=posiP, w0T=w0T, w1T=w1T, w2T=w2T,
                gb=np.ascontiguousarray(gb), wfT=wfT, woutT=woutT,
                identD=np.eye(128, dtype=ml_dtypes.bfloat16))


def kernel(**inputs) -> np.ndarray:
    if "nc" not in _CACHE:
        _CACHE["nc"] = _build_nc()
    nc = _CACHE["nc"]

    x = np.asarray(inputs["x"], np.float32)     # (64, 256, 14, 14)
    shared = _pack_shared(inputs)

    in_maps = []
    for c in range(NCORES):
        xc = x[BPC * c:BPC * (c + 1)]           # (8, 256, 14, 14)
        xT = np.ascontiguousarray(
            xc.transpose(1, 0, 2, 3).reshape(DIM, BPC * NIJ)
        ).astype(ml_dtypes.bfloat16)
        m = dict(shared)
        m["xT"] = xT
        in_maps.append(m)

    res = run_bass_kernel_spmd(nc, in_maps, list(range(NCORES)))
    outs = []
    for c in range(NCORES):
        o = res.results[c]["out_f"].reshape(DIM, BPC, HP, HP)
        outs.append(o.transpose(1, 0, 2, 3))
    return np.ascontiguousarray(np.concatenate(outs, axis=0), dtype=np.float32)
